# revision 13
# baseline (speedup 1.0000x reference)
"""DotHash GNN message-passing kernel for 8 Trainium2 NeuronCores.

Strategy (1D graph/data parallel, per the sharding hint):
- Node rows are sharded 8 ways.  The host relabels nodes (degree-balanced
  snake assignment) so every 128-row tile carries a near-equal number of
  adjacency edges, and pads the node count so every core owns the same even
  number of tiles.
- node_vectors are uploaded as bf16 shards and AllGathered on device into a
  per-core table.
- Each core computes its shard of one_hop = A @ (w*nv) and two_hop =
  A @ one_hop with a matmul-based segment sum: for each pair of 128-row
  tiles, dma_gather the needed source rows (lo/hi table halves keep the
  int16 gather indices in range), build a one-hot selection matrix S on the
  vector engine (iota compare against each edge slot's local row id), and
  accumulate S.T @ G in PSUM.  node_weight is folded into S for hop one.
- one_hop / two_hop shards are exchanged with AllGather so every core holds
  the full tables.
- Query edges are sharded 8 ways and sorted into 4 groups by which table
  half their endpoints fall in; each group's rows are fetched with one
  dma_gather per table and the four dot-product families are computed with
  whole-group tensor_tensor + tensor_reduce ops (the compiler config
  forbids dynamic offsets on vector ops, so everything is static).
All floating-point math happens on device (bf16 storage, fp32 accumulate);
the host only sorts/pads/wraps integer index streams and casts dtypes.
"""

import os
import sys

import numpy as np

for _p in ("/opt/trn_rl_repo", "/root/.axon_site/_ro/trn_rl_repo"):
    if os.path.isdir(_p) and _p not in sys.path:
        sys.path.insert(0, _p)

import ml_dtypes  # noqa: E402
import concourse.bass as bass  # noqa: E402
import concourse.bacc as bacc  # noqa: E402
import concourse.mybir as mybir  # noqa: E402
import concourse.tile as tile  # noqa: E402
from concourse.bass_utils import run_bass_kernel_spmd  # noqa: E402

NCORES = 8
P = 128
bf16 = mybir.dt.bfloat16
f32 = mybir.dt.float32
i16 = mybir.dt.int16

_CACHE = {}


def _patch_cc_flags():
    """neuronxcc's DataLocalityOpt pass crashes on this program at full
    scale (assert isinstance(load.tensor, NeuronLocalTensor)); skip it."""
    from concourse import compiler_utils
    flags = compiler_utils.get_compiler_flags()
    tflag = next((f for f in flags if f.startswith("--tensorizer-options=")), None)
    if tflag is not None and "DataLocalityOpt" not in tflag:
        compiler_utils.set_compiler_flags(
            flags + [tflag + " --skip-pass=DataLocalityOpt"])


def _wrap16(idx):
    """Pack an int16 index vector (len % 128 == 0) into the [16, n/16]
    wrapped layout that dma_gather expects (idx i at [i%16, i//16])."""
    return idx.reshape(-1, 16).T.astype(np.int16)


def _build_program(dim, npad, tiles_per_core, c_lo, c_hi, ch):
    """Build the SPMD bass program.  All sizes are compile-time constants.

    ch: padded chunk count per query group (same for all groups/cores).
    """
    half = npad // 2
    shard = tiles_per_core * P
    c_tot = c_lo + c_hi
    npairs = tiles_per_core // 2

    nc = bacc.Bacc("TRN2", target_bir_lowering=False, debug=False,
                   num_devices=NCORES, num_swdge_queues=1)

    nv_in = nc.dram_tensor("nv", [shard, dim], bf16, kind="ExternalInput")
    idx_lo_d = nc.dram_tensor("idx_lo", [16, tiles_per_core * c_lo * 8], i16, kind="ExternalInput")
    idx_hi_d = nc.dram_tensor("idx_hi", [16, tiles_per_core * c_hi * 8], i16, kind="ExternalInput")
    rl_d = nc.dram_tensor("rl", [P, tiles_per_core * c_tot], bf16, kind="ExternalInput")
    w_d = nc.dram_tensor("w", [P, tiles_per_core * c_tot], bf16, kind="ExternalInput")
    qidx_s_d = nc.dram_tensor("qidx_s", [16, 4 * ch * 8], i16, kind="ExternalInput")
    qidx_t_d = nc.dram_tensor("qidx_t", [16, 4 * ch * 8], i16, kind="ExternalInput")
    qsc_d = nc.dram_tensor("qsc", [P, 4 * 4 * ch], bf16, kind="ExternalInput")
    out_d = nc.dram_tensor("out", [4, P, 4 * ch], f32, kind="ExternalOutput")

    dbg_mode = os.environ.get("KDBG", "")
    dbg_d = nc.dram_tensor("dbg", [npad, dim], bf16, kind="ExternalOutput") if dbg_mode else None

    nv_bounce = nc.dram_tensor("nv_bounce", [shard, dim], bf16)
    nv_table = nc.dram_tensor("nv_table", [npad, dim], bf16)
    oh_bounce = nc.dram_tensor("oh_bounce", [shard, dim], bf16)
    oh_table = nc.dram_tensor("oh_table", [npad, dim], bf16)
    th_bounce = nc.dram_tensor("th_bounce", [shard, dim], bf16)
    th_table = nc.dram_tensor("th_table", [npad, dim], bf16)

    krep = int(os.environ.get("KREP", "1"))

    # idx arrays arrive as [16, X] (the dma_gather wrap layout); the Q7
    # ucode wants them replicated across all 128 partitions, so expand them
    # once into internal DRAM with a broadcast DMA, then load slices.
    idx_reps = {}
    for nm, src_t in (("idx_lo", idx_lo_d), ("idx_hi", idx_hi_d),
                      ("qidx_s", qidx_s_d), ("qidx_t", qidx_t_d)):
        xcols = src_t.shape[1]
        rep_t = nc.dram_tensor(f"{nm}_rep", [P, xcols], i16)
        idx_reps[nm] = rep_t

    def replicate_idx():
        for nm, src_t in (("idx_lo", idx_lo_d), ("idx_hi", idx_hi_d),
                          ("qidx_s", qidx_s_d), ("qidx_t", qidx_t_d)):
            xcols = src_t.shape[1]
            rep_t = idx_reps[nm]
            sap = src_t[:]
            rep_src = bass.AP(sap.tensor, sap.offset,
                              [[0, 8], list(sap.ap[0]), list(sap.ap[1])])
            nc.sync.dma_start(rep_t[:].rearrange("(a b) c -> a b c", a=8), rep_src)

    def load_idx(pool, tag, src_rep, col0, ncols):
        t = pool.tile([P, ncols], i16, tag=tag, name=tag)
        nc.sync.dma_start(t[:], src_rep[:, bass.ds(col0, ncols)])
        return t

    def spmm_phase(tc, table_lo, table_hi, shard_sb, weighted, iota_t,
                   meta_pool, g_pool, s_pool, psum_pool):
        def body(i):
            idx_lo = load_idx(meta_pool, "idxlo", idx_reps["idx_lo"], i * (2 * c_lo * 8), 2 * c_lo * 8)
            idx_hi = load_idx(meta_pool, "idxhi", idx_reps["idx_hi"], i * (2 * c_hi * 8), 2 * c_hi * 8)
            rl_t = meta_pool.tile([P, 2 * c_tot], bf16, tag="rl")
            nc.sync.dma_start(rl_t[:], rl_d[:, bass.ds(i * 2 * c_tot, 2 * c_tot)])

            g_lo = g_pool.tile([P, 2 * c_lo, dim], bf16, tag="glo", name="g_lo")
            nc.gpsimd.dma_gather(
                g_lo[:], table_lo, idx_lo[:], 2 * c_lo * P, 2 * c_lo * P, dim,
                single_packet=2 * c_lo * P <= 1024, queue_num=0)
            g_hi = g_pool.tile([P, 2 * c_hi, dim], bf16, tag="ghi", name="g_hi")
            nc.gpsimd.dma_gather(
                g_hi[:], table_hi, idx_hi[:], 2 * c_hi * P, 2 * c_hi * P, dim,
                single_packet=2 * c_hi * P <= 1024, queue_num=0)

            s = s_pool.tile([P, 2 * c_tot * P], bf16, tag="s")
            rl_ap = rl_t[:]
            nc.vector.tensor_tensor(
                out=s[:],
                in0=bass.AP(rl_ap.tensor, rl_ap.offset,
                            [rl_ap.ap[0], [1, 2 * c_tot], [0, P]]),
                in1=iota_t[:].rearrange("p (c m) -> p c m", c=2 * c_tot),
                op=mybir.AluOpType.is_equal)
            if weighted:
                w_t = meta_pool.tile([P, 2 * c_tot], bf16, tag="w")
                nc.sync.dma_start(w_t[:], w_d[:, bass.ds(i * 2 * c_tot, 2 * c_tot)])
                w_ap = w_t[:]
                nc.vector.tensor_tensor(
                    out=s[:],
                    in0=s[:].rearrange("p (c m) -> p c m", c=2 * c_tot),
                    in1=bass.AP(w_ap.tensor, w_ap.offset,
                                [w_ap.ap[0], [1, 2 * c_tot], [0, P]]),
                    op=mybir.AluOpType.mult)

            # pair-half h (tile 2i+h) uses S chunks h*c_tot + cc; its lo
            # chunks sit at g_lo[:, h*c_lo + cc], hi at g_hi[:, h*c_hi + ...].
            for h in range(2):
                ps = psum_pool.tile([P, dim], f32, tag="ps")
                for cc in range(c_tot):
                    if cc < c_lo:
                        g_ap = g_lo[:, h * c_lo + cc, :]
                    else:
                        g_ap = g_hi[:, h * c_hi + (cc - c_lo), :]
                    sc = (h * c_tot + cc) * P
                    nc.tensor.matmul(ps[:], s[:, sc:sc + P], g_ap,
                                     start=(cc == 0), stop=(cc == c_tot - 1))
                nc.scalar.copy(shard_sb[:, bass.ds(2 * i + h, 1), :], ps[:, None, :])

        for _ in range(krep):
            tc.For_i_unrolled(0, npairs, 1, body, max_unroll=2)

    with tile.TileContext(nc) as tc:
        with (
            tc.tile_pool(name="const", bufs=1) as const_pool,
        ):
            iota_t = const_pool.tile([P, 2 * c_tot * P], bf16)
            nc.gpsimd.iota(iota_t[:], pattern=[[0, 2 * c_tot], [1, P]], base=0,
                           channel_multiplier=0, allow_small_or_imprecise_dtypes=True)

            # ---- phase 0: replicate idx arrays, distribute node vectors ----
            replicate_idx()
            nc.sync.dma_start(nv_bounce[:], nv_in[:])
            nc.gpsimd.collective_compute(
                "AllGather", mybir.AluOpType.bypass,
                replica_groups=[list(range(NCORES))],
                ins=[nv_bounce[:]], outs=[nv_table[:]])

            # ---- phase A: one_hop shard ----
            with (
                tc.tile_pool(name="shardA", bufs=1) as shard_pool,
                tc.tile_pool(name="metaA", bufs=3) as meta_pool,
                tc.tile_pool(name="gA", bufs=2) as g_pool,
                tc.tile_pool(name="sA", bufs=2) as s_pool,
                tc.tile_pool(name="psA", bufs=2, space="PSUM") as psum_pool,
            ):
                oh_sb = shard_pool.tile([P, tiles_per_core, dim], bf16)
                spmm_phase(tc, nv_table[0:half, :], nv_table[half:npad, :], oh_sb, True,
                           iota_t, meta_pool, g_pool, s_pool, psum_pool)
                nc.sync.dma_start(oh_bounce[:].rearrange("(t p) d -> p t d", p=P), oh_sb[:])
            if dbg_mode == "A":
                nc.sync.dma_start(dbg_d[0:shard, :], oh_bounce[:])
            if dbg_mode != "A":
                nc.gpsimd.collective_compute(
                    "AllGather", mybir.AluOpType.bypass,
                    replica_groups=[list(range(NCORES))],
                    ins=[oh_bounce[:]], outs=[oh_table[:]])
                if dbg_mode == "AG":
                    nc.sync.dma_start(dbg_d[:], oh_table[:])

            # ---- phase B: two_hop shard ----
            if dbg_mode not in ("A", "AG"):
                with (
                    tc.tile_pool(name="shardB", bufs=1) as shard_pool,
                    tc.tile_pool(name="metaB", bufs=3) as meta_pool,
                    tc.tile_pool(name="gB", bufs=2) as g_pool,
                    tc.tile_pool(name="sB", bufs=2) as s_pool,
                    tc.tile_pool(name="psB", bufs=2, space="PSUM") as psum_pool,
                ):
                    th_sb = shard_pool.tile([P, tiles_per_core, dim], bf16)
                    spmm_phase(tc, oh_table[0:half, :], oh_table[half:npad, :], th_sb, False,
                               iota_t, meta_pool, g_pool, s_pool, psum_pool)
                    nc.sync.dma_start(th_bounce[:].rearrange("(t p) d -> p t d", p=P), th_sb[:])
                nc.gpsimd.collective_compute(
                    "AllGather", mybir.AluOpType.bypass,
                    replica_groups=[list(range(NCORES))],
                    ins=[th_bounce[:]], outs=[th_table[:]])
                if dbg_mode == "AB":
                    nc.sync.dma_start(dbg_d[:], th_table[:])

            # ---- phase C: query dots (no loops; whole-group tensors) ----
            if dbg_mode == "":
                with (
                    tc.tile_pool(name="qidx", bufs=2) as qidx_pool,
                    tc.tile_pool(name="qg", bufs=1) as qg_pool,
                    tc.tile_pool(name="qtmp", bufs=1) as qtmp_pool,
                    tc.tile_pool(name="qout", bufs=1) as qout_pool,
                ):
                    mul = mybir.AluOpType.mult
                    add = mybir.AluOpType.add
                    sub = mybir.AluOpType.subtract
                    X = mybir.AxisListType.X
                    nidx = ch * P
                    for _ in range(krep):
                        for g in range(4):
                            s_lo = (g // 2) == 0
                            t_lo = (g % 2) == 0

                            def tab(t_, lo):
                                return t_[0:half, :] if lo else t_[half:npad, :]

                            idx_s = load_idx(qidx_pool, "qis", idx_reps["qidx_s"], g * ch * 8, ch * 8)
                            idx_t = load_idx(qidx_pool, "qit", idx_reps["qidx_t"], g * ch * 8, ch * 8)
                            sc_b = qidx_pool.tile([P, 4, ch], bf16, tag="scb", name="sc_b")
                            nc.sync.dma_start(sc_b[:], qsc_d[:, g * 4 * ch:(g + 1) * 4 * ch]
                                              .rearrange("p (j c) -> p j c", j=4))
                            cs_b = qidx_pool.tile([P, ch], f32, tag="csb", name="cs_b")
                            ct_b = qidx_pool.tile([P, ch], f32, tag="ctb", name="ct_b")
                            nc.vector.tensor_tensor(out=cs_b[:], in0=sc_b[:, 0, :],
                                                    in1=sc_b[:, 1, :], op=mul)
                            nc.vector.tensor_tensor(out=ct_b[:], in0=sc_b[:, 2, :],
                                                    in1=sc_b[:, 3, :], op=mul)

                            tiles = {}
                            for name, table, idxt in (
                                    ("ohs", tab(oh_table, s_lo), idx_s),
                                    ("oht", tab(oh_table, t_lo), idx_t),
                                    ("ths", tab(th_table, s_lo), idx_s),
                                    ("tht", tab(th_table, t_lo), idx_t),
                                    ("nvs", tab(nv_table, s_lo), idx_s),
                                    ("nvt", tab(nv_table, t_lo), idx_t)):
                                t_ = qg_pool.tile([P, ch, dim], bf16, tag=name, name=name)
                                nc.gpsimd.dma_gather(t_[:], table, idxt[:], nidx, nidx,
                                                     dim, single_packet=nidx <= 1024,
                                                     queue_num=0)
                                tiles[name] = t_

                            acc = qout_pool.tile([P, 6, ch], f32, tag="acc", name="acc")
                            prod = qtmp_pool.tile([P, ch, dim], bf16, tag="prod", name="prod")
                            zs_t = qtmp_pool.tile([P, ch, dim], bf16, tag="zs", name="zs_t")
                            zt_t = qtmp_pool.tile([P, ch, dim], bf16, tag="zt", name="zt_t")

                            def bcast(t2d):
                                ap = t2d[:]
                                return bass.AP(ap.tensor, ap.offset,
                                               [ap.ap[0], [1, ch], [0, dim]])

                            def dot(dst_j, a_ap, b_ap):
                                nc.vector.tensor_tensor(out=prod[:], in0=a_ap, in1=b_ap, op=mul)
                                nc.vector.tensor_reduce(out=acc[:, dst_j, :], in_=prod[:],
                                                        axis=X, op=add)

                            ohs, oht = tiles["ohs"][:], tiles["oht"][:]
                            ths, tht = tiles["ths"][:], tiles["tht"][:]
                            dot(0, ohs, oht)
                            dot(1, ohs, tht)
                            dot(2, ths, oht)
                            dot(4, ohs, ths)
                            dot(5, oht, tht)
                            # z = th - (deg*w) * nv
                            nc.vector.tensor_tensor(out=zs_t[:], in0=tiles["nvs"][:],
                                                    in1=bcast(cs_b), op=mul)
                            nc.vector.tensor_tensor(out=zs_t[:], in0=ths, in1=zs_t[:], op=sub)
                            nc.vector.tensor_tensor(out=zt_t[:], in0=tiles["nvt"][:],
                                                    in1=bcast(ct_b), op=mul)
                            nc.vector.tensor_tensor(out=zt_t[:], in0=tht, in1=zt_t[:], op=sub)
                            dot(3, zs_t[:], zt_t[:])
                            # c12 = acc1+acc2, cself = acc4+acc5
                            nc.vector.tensor_tensor(out=acc[:, 1, :], in0=acc[:, 1, :],
                                                    in1=acc[:, 2, :], op=add)
                            nc.vector.tensor_tensor(out=acc[:, 4, :], in0=acc[:, 4, :],
                                                    in1=acc[:, 5, :], op=add)
                            for jj, aj in enumerate((0, 1, 3, 4)):
                                nc.sync.dma_start(out_d[jj][:, g * ch:(g + 1) * ch],
                                                  acc[:, aj, :])

    nc.compile()
    return nc


def _prepare(edges, adj_row, adj_col, node_weight, node_vectors):
    edges = np.asarray(edges)
    adj_row = np.asarray(adj_row).astype(np.int64)
    adj_col = np.asarray(adj_col).astype(np.int64)
    node_weight = np.asarray(node_weight, dtype=np.float32)
    node_vectors = np.asarray(node_vectors, dtype=np.float32)

    n, dim = node_vectors.shape
    eq = edges.shape[1]
    s_nodes = np.asarray(edges[0]).astype(np.int64)
    t_nodes = np.asarray(edges[1]).astype(np.int64)

    tiles_per_core = -(-n // (NCORES * P))
    tiles_per_core += tiles_per_core % 2  # even, for pair-gathers
    shard = tiles_per_core * P
    npad = NCORES * shard
    half = npad // 2
    ntiles = NCORES * tiles_per_core
    assert half <= 32767, "table half must fit int16 gather indices"

    deg = np.bincount(adj_row, minlength=n).astype(np.float32)

    # degree-balanced relabeling: snake rows (sorted by degree desc) across
    # all tiles so each tile carries ~the same number of edges.
    order_rows = np.argsort(-deg, kind="stable")
    slot_ids = np.arange(npad)
    rounds = slot_ids // ntiles                    # 0..127 (= row slot in tile)
    pos = slot_ids % ntiles
    tiles_seq = np.where(rounds % 2 == 0, pos, ntiles - 1 - pos)
    new_ids_seq = tiles_seq * P + rounds           # new id for degree-rank r
    perm = np.full(npad, -1, np.int64)             # new_id -> old_id
    perm[new_ids_seq[:n]] = order_rows
    valid = perm >= 0
    pi = np.full(n, -1, np.int64)                  # old_id -> new_id
    pi[perm[valid]] = np.nonzero(valid)[0]

    row_new = pi[adj_row]
    col_new = pi[adj_col]
    s_new = pi[s_nodes]
    t_new = pi[t_nodes]

    w_bf = node_weight.astype(ml_dtypes.bfloat16)
    nv_pad = np.zeros((npad, dim), ml_dtypes.bfloat16)
    nv_pad[valid] = node_vectors.astype(ml_dtypes.bfloat16)[perm[valid]]

    core_of = row_new // shard
    tile_of = (row_new % shard) // P
    rl_of = row_new % P
    is_lo = col_new < half

    key = core_of * tiles_per_core + tile_of
    cnt_lo = np.bincount(key[is_lo], minlength=ntiles)
    cnt_hi = np.bincount(key[~is_lo], minlength=ntiles)
    c_lo = max(1, int(-(-cnt_lo.max() // P)))
    c_hi = max(1, int(-(-cnt_hi.max() // P)))
    c_tot = c_lo + c_hi

    order = np.lexsort((~is_lo, tile_of, core_of))

    # ---- query groups ----
    q_core = np.repeat(np.arange(NCORES), -(-eq // NCORES))[:eq]
    q_group = np.where(s_new < half, 0, 2) + np.where(t_new < half, 0, 1)
    grp_cnt = np.zeros((NCORES, 4), np.int64)
    for k in range(NCORES):
        m = q_core == k
        grp_cnt[k] = np.bincount(q_group[m], minlength=4)
    ch = max(1, int(-(-grp_cnt.max() // P)))

    cache_key = (dim, npad, tiles_per_core, c_lo, c_hi, ch)
    if cache_key not in _CACHE:
        _CACHE[cache_key] = _build_program(dim, npad, tiles_per_core, c_lo, c_hi, ch)
    nc = _CACHE[cache_key]

    wcol_bf = w_bf[adj_col].astype(np.float32)
    deg_new = np.zeros(npad, np.float32)
    deg_new[valid] = deg[perm[valid]]
    w_new = np.zeros(npad, np.float32)
    w_new[valid] = w_bf[perm[valid]].astype(np.float32)

    in_maps = []
    q_positions = []
    for k in range(NCORES):
        sel = order[core_of[order] == k]
        idx_lo_arr = np.zeros((tiles_per_core, c_lo * P), np.int16)
        idx_hi_arr = np.zeros((tiles_per_core, c_hi * P), np.int16)
        rl_arr = np.full((P, tiles_per_core * c_tot), 255.0, np.float32)
        w_arr = np.zeros((P, tiles_per_core * c_tot), np.float32)
        for t in range(tiles_per_core):
            et = sel[tile_of[sel] == t]
            lo_e = et[is_lo[et]]
            hi_e = et[~is_lo[et]]
            nl, nh = len(lo_e), len(hi_e)
            idx_lo_arr[t, :nl] = col_new[lo_e]
            idx_hi_arr[t, :nh] = col_new[hi_e] - half
            slots = np.arange(nl)
            rl_arr[slots % P, t * c_tot + slots // P] = rl_of[lo_e]
            w_arr[slots % P, t * c_tot + slots // P] = wcol_bf[lo_e]
            slots = np.arange(nh)
            rl_arr[slots % P, t * c_tot + c_lo + slots // P] = rl_of[hi_e]
            w_arr[slots % P, t * c_tot + c_lo + slots // P] = wcol_bf[hi_e]

        idx_lo_w = np.concatenate([_wrap16(idx_lo_arr[t]) for t in range(tiles_per_core)], axis=1)
        idx_hi_w = np.concatenate([_wrap16(idx_hi_arr[t]) for t in range(tiles_per_core)], axis=1)

        qsel = np.nonzero(q_core == k)[0]
        qidx_s_arr = np.zeros((4, ch * P), np.int16)
        qidx_t_arr = np.zeros((4, ch * P), np.int16)
        qsc_arr = np.zeros((P, 4 * 4 * ch), np.float32)
        qpos = np.full((4, ch * P), -1, np.int64)
        for g in range(4):
            qg = qsel[q_group[qsel] == g]
            m = len(qg)
            sv = s_new[qg]
            tv = t_new[qg]
            qidx_s_arr[g, :m] = np.where(sv < half, sv, sv - half)
            qidx_t_arr[g, :m] = np.where(tv < half, tv, tv - half)
            qpos[g, :m] = qg
            slots = np.arange(m)
            pcol = (slots % P, slots // P)
            base = g * 4 * ch
            qsc_arr[pcol[0], base + pcol[1]] = deg_new[sv]
            qsc_arr[pcol[0], base + ch + pcol[1]] = w_new[sv]
            qsc_arr[pcol[0], base + 2 * ch + pcol[1]] = deg_new[tv]
            qsc_arr[pcol[0], base + 3 * ch + pcol[1]] = w_new[tv]

        qidx_s_w = np.concatenate([_wrap16(qidx_s_arr[g]) for g in range(4)], axis=1)
        qidx_t_w = np.concatenate([_wrap16(qidx_t_arr[g]) for g in range(4)], axis=1)

        in_maps.append({
            "nv": np.ascontiguousarray(nv_pad[k * shard:(k + 1) * shard]),
            "idx_lo": idx_lo_w,
            "idx_hi": idx_hi_w,
            "rl": rl_arr.astype(ml_dtypes.bfloat16),
            "w": w_arr.astype(ml_dtypes.bfloat16),
            "qidx_s": qidx_s_w,
            "qidx_t": qidx_t_w,
            "qsc": qsc_arr.astype(ml_dtypes.bfloat16),
        })
        q_positions.append(qpos)

    return nc, in_maps, q_positions, eq, ch


def kernel(edges, adj_row, adj_col, node_weight, node_vectors):
    _patch_cc_flags()
    nc, in_maps, q_positions, eq, ch = _prepare(
        edges, adj_row, adj_col, node_weight, node_vectors)
    res = run_bass_kernel_spmd(nc, in_maps, core_ids=list(range(NCORES)))
    outs = [res.results[k]["out"] for k in range(NCORES)]
    return _assemble(outs, q_positions, eq, ch)


def _assemble(outs, q_positions, eq, ch):
    counts = [np.zeros(eq, np.float32) for _ in range(4)]
    for k in range(NCORES):
        out = outs[k]  # [4, 128, 4*ch]
        for g in range(4):
            qpos = q_positions[k][g]
            slots = np.nonzero(qpos >= 0)[0]
            pp = slots % P
            cc = g * ch + slots // P
            for j in range(4):
                counts[j][qpos[slots]] = out[j, pp, cc]
    return tuple(counts)


# revision 16
# speedup vs baseline: 1.0319x; 1.0319x over previous
"""DotHash GNN message-passing kernel for 8 Trainium2 NeuronCores.

Strategy (1D graph/data parallel, per the sharding hint):
- Node rows are sharded 8 ways.  The host relabels nodes (degree-balanced
  snake assignment) so every 128-row tile carries a near-equal number of
  adjacency edges, and pads the node count so every core owns the same even
  number of tiles.
- node_vectors are uploaded as bf16 shards and AllGathered on device into a
  per-core table.
- Each core computes its shard of one_hop = A @ (w*nv) and two_hop =
  A @ one_hop with a matmul-based segment sum: for each pair of 128-row
  tiles, dma_gather the needed source rows (lo/hi table halves keep the
  int16 gather indices in range), build a one-hot selection matrix S on the
  vector engine (iota compare against each edge slot's local row id), and
  accumulate S.T @ G in PSUM.  node_weight is folded into S for hop one.
- one_hop / two_hop shards are exchanged with AllGather so every core holds
  the full tables.
- Query edges are sharded 8 ways and sorted into 4 groups by which table
  half their endpoints fall in; each group's rows are fetched with one
  dma_gather per table and the four dot-product families are computed with
  whole-group tensor_tensor + tensor_reduce ops (the compiler config
  forbids dynamic offsets on vector ops, so everything is static).
All floating-point math happens on device (bf16 storage, fp32 accumulate);
the host only sorts/pads/wraps integer index streams and casts dtypes.
"""

import os
import sys

import numpy as np

for _p in ("/opt/trn_rl_repo", "/root/.axon_site/_ro/trn_rl_repo"):
    if os.path.isdir(_p) and _p not in sys.path:
        sys.path.insert(0, _p)

import ml_dtypes  # noqa: E402
import concourse.bass as bass  # noqa: E402
import concourse.bacc as bacc  # noqa: E402
import concourse.mybir as mybir  # noqa: E402
import concourse.tile as tile  # noqa: E402
from concourse.bass_utils import run_bass_kernel_spmd  # noqa: E402

NCORES = 8
P = 128
bf16 = mybir.dt.bfloat16
f32 = mybir.dt.float32
i16 = mybir.dt.int16

_CACHE = {}


def _patch_cc_flags():
    """neuronxcc's DataLocalityOpt pass crashes on this program at full
    scale (assert isinstance(load.tensor, NeuronLocalTensor)); skip it."""
    from concourse import compiler_utils
    flags = compiler_utils.get_compiler_flags()
    tflag = next((f for f in flags if f.startswith("--tensorizer-options=")), None)
    if tflag is not None and "DataLocalityOpt" not in tflag:
        compiler_utils.set_compiler_flags(
            flags + [tflag + " --skip-pass=DataLocalityOpt"])


def _wrap16(idx):
    """Pack an int16 index vector (len % 128 == 0) into the [16, n/16]
    wrapped layout that dma_gather expects (idx i at [i%16, i//16])."""
    return idx.reshape(-1, 16).T.astype(np.int16)


def _build_program(dim, npad, tiles_per_core, c_lo, c_hi, ch):
    """Build the SPMD bass program.  All sizes are compile-time constants.

    ch: padded chunk count per query group (same for all groups/cores).
    """
    half = npad // 2
    shard = tiles_per_core * P
    c_tot = c_lo + c_hi
    npairs = tiles_per_core // 2

    nc = bacc.Bacc("TRN2", target_bir_lowering=False, debug=False,
                   num_devices=NCORES, num_swdge_queues=1)

    nv_in = nc.dram_tensor("nv", [shard, dim], bf16, kind="ExternalInput")
    idx_lo_d = nc.dram_tensor("idx_lo", [16, tiles_per_core * c_lo * 8], i16, kind="ExternalInput")
    idx_hi_d = nc.dram_tensor("idx_hi", [16, tiles_per_core * c_hi * 8], i16, kind="ExternalInput")
    rl_d = nc.dram_tensor("rl", [P, tiles_per_core * c_tot], bf16, kind="ExternalInput")
    w_d = nc.dram_tensor("w", [P, tiles_per_core * c_tot], bf16, kind="ExternalInput")
    qidx_s_d = nc.dram_tensor("qidx_s", [16, 4 * ch * 8], i16, kind="ExternalInput")
    qidx_t_d = nc.dram_tensor("qidx_t", [16, 4 * ch * 8], i16, kind="ExternalInput")
    qsc_d = nc.dram_tensor("qsc", [P, 4 * 4 * ch], bf16, kind="ExternalInput")
    out_d = nc.dram_tensor("out", [4, P, 4 * ch], f32, kind="ExternalOutput")

    dbg_mode = os.environ.get("KDBG", "")
    dbg_d = nc.dram_tensor("dbg", [npad, dim], bf16, kind="ExternalOutput") if dbg_mode else None

    nv_bounce = nc.dram_tensor("nv_bounce", [shard, dim], bf16)
    nv_table = nc.dram_tensor("nv_table", [npad, dim], bf16, addr_space="Shared")
    oh_bounce = nc.dram_tensor("oh_bounce", [shard, dim], bf16)
    oh_table = nc.dram_tensor("oh_table", [npad, dim], bf16, addr_space="Shared")
    comb_bounce = nc.dram_tensor("comb_bounce", [shard, 2 * dim], bf16)
    comb_table = nc.dram_tensor("comb_table", [npad, 2 * dim], bf16, addr_space="Shared")

    krep = int(os.environ.get("KREP", "1"))

    # idx arrays arrive as [16, X] (the dma_gather wrap layout); the Q7
    # ucode wants them replicated across all 128 partitions, so expand them
    # once into internal DRAM with a broadcast DMA, then load slices.
    idx_reps = {}
    for nm, src_t in (("idx_lo", idx_lo_d), ("idx_hi", idx_hi_d),
                      ("qidx_s", qidx_s_d), ("qidx_t", qidx_t_d)):
        xcols = src_t.shape[1]
        rep_t = nc.dram_tensor(f"{nm}_rep", [P, xcols], i16)
        idx_reps[nm] = rep_t

    def replicate_idx():
        for nm, src_t in (("idx_lo", idx_lo_d), ("idx_hi", idx_hi_d),
                          ("qidx_s", qidx_s_d), ("qidx_t", qidx_t_d)):
            xcols = src_t.shape[1]
            rep_t = idx_reps[nm]
            sap = src_t[:]
            rep_src = bass.AP(sap.tensor, sap.offset,
                              [[0, 8], list(sap.ap[0]), list(sap.ap[1])])
            nc.sync.dma_start(rep_t[:].rearrange("(a b) c -> a b c", a=8), rep_src)

    def load_idx(pool, tag, src_rep, col0, ncols):
        t = pool.tile([P, ncols], i16, tag=tag, name=tag)
        nc.sync.dma_start(t[:], src_rep[:, bass.ds(col0, ncols)])
        return t

    def spmm_phase(tc, table_lo, table_hi, shard_sb, weighted, iota_t,
                   meta_pool, g_pool, s_pool, psum_pool):
        def body(i):
            idx_lo = load_idx(meta_pool, "idxlo", idx_reps["idx_lo"], i * (2 * c_lo * 8), 2 * c_lo * 8)
            idx_hi = load_idx(meta_pool, "idxhi", idx_reps["idx_hi"], i * (2 * c_hi * 8), 2 * c_hi * 8)
            rl_t = meta_pool.tile([P, 2 * c_tot], bf16, tag="rl")
            nc.sync.dma_start(rl_t[:], rl_d[:, bass.ds(i * 2 * c_tot, 2 * c_tot)])

            g_lo = g_pool.tile([P, 2 * c_lo, dim], bf16, tag="glo", name="g_lo")
            nc.gpsimd.dma_gather(
                g_lo[:], table_lo, idx_lo[:], 2 * c_lo * P, 2 * c_lo * P, dim,
                single_packet=2 * c_lo * P <= 1024, queue_num=0)
            g_hi = g_pool.tile([P, 2 * c_hi, dim], bf16, tag="ghi", name="g_hi")
            nc.gpsimd.dma_gather(
                g_hi[:], table_hi, idx_hi[:], 2 * c_hi * P, 2 * c_hi * P, dim,
                single_packet=2 * c_hi * P <= 1024, queue_num=0)

            s = s_pool.tile([P, 2 * c_tot * P], bf16, tag="s")
            rl_ap = rl_t[:]
            nc.vector.tensor_tensor(
                out=s[:],
                in0=bass.AP(rl_ap.tensor, rl_ap.offset,
                            [rl_ap.ap[0], [1, 2 * c_tot], [0, P]]),
                in1=iota_t[:].rearrange("p (c m) -> p c m", c=2 * c_tot),
                op=mybir.AluOpType.is_equal)
            if weighted:
                w_t = meta_pool.tile([P, 2 * c_tot], bf16, tag="w")
                nc.sync.dma_start(w_t[:], w_d[:, bass.ds(i * 2 * c_tot, 2 * c_tot)])
                w_ap = w_t[:]
                nc.vector.tensor_tensor(
                    out=s[:],
                    in0=s[:].rearrange("p (c m) -> p c m", c=2 * c_tot),
                    in1=bass.AP(w_ap.tensor, w_ap.offset,
                                [w_ap.ap[0], [1, 2 * c_tot], [0, P]]),
                    op=mybir.AluOpType.mult)

            # pair-half h (tile 2i+h) uses S chunks h*c_tot + cc; its lo
            # chunks sit at g_lo[:, h*c_lo + cc], hi at g_hi[:, h*c_hi + ...].
            for h in range(2):
                ps = psum_pool.tile([P, dim], f32, tag="ps")
                for cc in range(c_tot):
                    if cc < c_lo:
                        g_ap = g_lo[:, h * c_lo + cc, :]
                    else:
                        g_ap = g_hi[:, h * c_hi + (cc - c_lo), :]
                    sc = (h * c_tot + cc) * P
                    nc.tensor.matmul(ps[:], s[:, sc:sc + P], g_ap,
                                     start=(cc == 0), stop=(cc == c_tot - 1))
                nc.scalar.copy(shard_sb[:, bass.ds(2 * i + h, 1), :], ps[:, None, :])

        for _ in range(krep):
            tc.For_i_unrolled(0, npairs, 1, body, max_unroll=2)

    with tile.TileContext(nc) as tc:
        with (
            tc.tile_pool(name="const", bufs=1) as const_pool,
        ):
            iota_t = const_pool.tile([P, 2 * c_tot * P], bf16)
            nc.gpsimd.iota(iota_t[:], pattern=[[0, 2 * c_tot], [1, P]], base=0,
                           channel_multiplier=0, allow_small_or_imprecise_dtypes=True)

            # ---- phase 0: replicate idx arrays, distribute node vectors ----
            replicate_idx()
            nc.sync.dma_start(nv_bounce[:], nv_in[:])
            nc.gpsimd.collective_compute(
                "AllGather", mybir.AluOpType.bypass,
                replica_groups=[list(range(NCORES))],
                ins=[nv_bounce[:]], outs=[nv_table[:]])

            # ---- phase A: one_hop shard ----
            with (
                tc.tile_pool(name="shardA", bufs=1) as shard_pool,
                tc.tile_pool(name="metaA", bufs=3) as meta_pool,
                tc.tile_pool(name="gA", bufs=2) as g_pool,
                tc.tile_pool(name="sA", bufs=2) as s_pool,
                tc.tile_pool(name="psA", bufs=2, space="PSUM") as psum_pool,
            ):
                oh_sb = shard_pool.tile([P, tiles_per_core, dim], bf16)
                spmm_phase(tc, nv_table[0:half, :], nv_table[half:npad, :], oh_sb, True,
                           iota_t, meta_pool, g_pool, s_pool, psum_pool)
                nc.sync.dma_start(oh_bounce[:].rearrange("(t p) d -> p t d", p=P), oh_sb[:])
            if dbg_mode == "A":
                nc.sync.dma_start(dbg_d[0:shard, :], oh_bounce[:])
            if dbg_mode != "A":
                nc.gpsimd.collective_compute(
                    "AllGather", mybir.AluOpType.bypass,
                    replica_groups=[list(range(NCORES))],
                    ins=[oh_bounce[:]], outs=[oh_table[:]])
                if dbg_mode == "AG":
                    nc.sync.dma_start(dbg_d[:], oh_table[:])

            # ---- phase B: two_hop shard ----
            if dbg_mode not in ("A", "AG"):
                with (
                    tc.tile_pool(name="shardB", bufs=1) as shard_pool,
                    tc.tile_pool(name="metaB", bufs=3) as meta_pool,
                    tc.tile_pool(name="gB", bufs=2) as g_pool,
                    tc.tile_pool(name="sB", bufs=2) as s_pool,
                    tc.tile_pool(name="psB", bufs=2, space="PSUM") as psum_pool,
                ):
                    th_sb = shard_pool.tile([P, tiles_per_core, dim], bf16)
                    spmm_phase(tc, oh_table[0:half, :], oh_table[half:npad, :], th_sb, False,
                               iota_t, meta_pool, g_pool, s_pool, psum_pool)
                    # interleave [oh | th] per row so the query phase can fetch
                    # both with one 1KB-row gather (cheaper than 2x512B).
                    nc.sync.dma_start(
                        comb_bounce[:, dim:2 * dim].rearrange("(t p) d -> p t d", p=P),
                        th_sb[:])
                nc.sync.dma_start(comb_bounce[:, 0:dim], oh_bounce[:])
                nc.gpsimd.collective_compute(
                    "AllGather", mybir.AluOpType.bypass,
                    replica_groups=[list(range(NCORES))],
                    ins=[comb_bounce[:]], outs=[comb_table[:]])
                if dbg_mode == "AB":
                    nc.sync.dma_start(dbg_d[:], comb_table[:, dim:2 * dim])

            # ---- phase C: query dots (no loops; whole-group tensors) ----
            if dbg_mode == "":
                with (
                    tc.tile_pool(name="qidx", bufs=2) as qidx_pool,
                    tc.tile_pool(name="qg", bufs=1) as qg_pool,
                    tc.tile_pool(name="qtmp", bufs=1) as qtmp_pool,
                    tc.tile_pool(name="qout", bufs=1) as qout_pool,
                ):
                    mul = mybir.AluOpType.mult
                    add = mybir.AluOpType.add
                    sub = mybir.AluOpType.subtract
                    X = mybir.AxisListType.X
                    nidx = ch * P
                    for _ in range(krep):
                        for g in range(4):
                            s_lo = (g // 2) == 0
                            t_lo = (g % 2) == 0

                            def tab(t_, lo):
                                return t_[0:half, :] if lo else t_[half:npad, :]

                            idx_s = load_idx(qidx_pool, "qis", idx_reps["qidx_s"], g * ch * 8, ch * 8)
                            idx_t = load_idx(qidx_pool, "qit", idx_reps["qidx_t"], g * ch * 8, ch * 8)
                            sc_b = qidx_pool.tile([P, 4, ch], bf16, tag="scb", name="sc_b")
                            nc.sync.dma_start(sc_b[:], qsc_d[:, g * 4 * ch:(g + 1) * 4 * ch]
                                              .rearrange("p (j c) -> p j c", j=4))
                            cs_b = qidx_pool.tile([P, ch], f32, tag="csb", name="cs_b")
                            ct_b = qidx_pool.tile([P, ch], f32, tag="ctb", name="ct_b")
                            nc.vector.tensor_tensor(out=cs_b[:], in0=sc_b[:, 0, :],
                                                    in1=sc_b[:, 1, :], op=mul)
                            nc.vector.tensor_tensor(out=ct_b[:], in0=sc_b[:, 2, :],
                                                    in1=sc_b[:, 3, :], op=mul)

                            tiles = {}
                            for name, table, idxt in (
                                    ("cs", tab(comb_table, s_lo), idx_s),
                                    ("ct", tab(comb_table, t_lo), idx_t)):
                                t_ = qg_pool.tile([P, ch, 2 * dim], bf16, tag=name, name=name)
                                nc.gpsimd.dma_gather(t_[:], table, idxt[:], nidx, nidx,
                                                     2 * dim, single_packet=nidx <= 1024,
                                                     queue_num=0)
                                tiles[name] = t_
                            for name, table, idxt in (
                                    ("nvs", tab(nv_table, s_lo), idx_s),
                                    ("nvt", tab(nv_table, t_lo), idx_t)):
                                t_ = qg_pool.tile([P, ch, dim], bf16, tag=name, name=name)
                                nc.gpsimd.dma_gather(t_[:], table, idxt[:], nidx, nidx,
                                                     dim, single_packet=nidx <= 1024,
                                                     queue_num=0)
                                tiles[name] = t_

                            acc = qout_pool.tile([P, 6, ch], f32, tag="acc", name="acc")
                            prod = qtmp_pool.tile([P, ch, dim], bf16, tag="prod", name="prod")
                            zs_t = qtmp_pool.tile([P, ch, dim], bf16, tag="zs", name="zs_t")
                            zt_t = qtmp_pool.tile([P, ch, dim], bf16, tag="zt", name="zt_t")

                            def bcast(t2d):
                                ap = t2d[:]
                                return bass.AP(ap.tensor, ap.offset,
                                               [ap.ap[0], [1, ch], [0, dim]])

                            def dot(dst_j, a_ap, b_ap):
                                nc.vector.tensor_tensor(out=prod[:], in0=a_ap, in1=b_ap, op=mul)
                                nc.vector.tensor_reduce(out=acc[:, dst_j, :], in_=prod[:],
                                                        axis=X, op=add)

                            ohs = tiles["cs"][:, :, 0:dim]
                            ths = tiles["cs"][:, :, dim:2 * dim]
                            oht = tiles["ct"][:, :, 0:dim]
                            tht = tiles["ct"][:, :, dim:2 * dim]
                            dot(0, ohs, oht)
                            dot(1, ohs, tht)
                            dot(2, ths, oht)
                            dot(4, ohs, ths)
                            dot(5, oht, tht)
                            # z = th - (deg*w) * nv
                            nc.vector.tensor_tensor(out=zs_t[:], in0=tiles["nvs"][:],
                                                    in1=bcast(cs_b), op=mul)
                            nc.vector.tensor_tensor(out=zs_t[:], in0=ths, in1=zs_t[:], op=sub)
                            nc.vector.tensor_tensor(out=zt_t[:], in0=tiles["nvt"][:],
                                                    in1=bcast(ct_b), op=mul)
                            nc.vector.tensor_tensor(out=zt_t[:], in0=tht, in1=zt_t[:], op=sub)
                            dot(3, zs_t[:], zt_t[:])
                            # c12 = acc1+acc2, cself = acc4+acc5
                            nc.vector.tensor_tensor(out=acc[:, 1, :], in0=acc[:, 1, :],
                                                    in1=acc[:, 2, :], op=add)
                            nc.vector.tensor_tensor(out=acc[:, 4, :], in0=acc[:, 4, :],
                                                    in1=acc[:, 5, :], op=add)
                            for jj, aj in enumerate((0, 1, 3, 4)):
                                nc.sync.dma_start(out_d[jj][:, g * ch:(g + 1) * ch],
                                                  acc[:, aj, :])

    nc.compile()
    return nc


def _prepare(edges, adj_row, adj_col, node_weight, node_vectors):
    edges = np.asarray(edges)
    adj_row = np.asarray(adj_row).astype(np.int64)
    adj_col = np.asarray(adj_col).astype(np.int64)
    node_weight = np.asarray(node_weight, dtype=np.float32)
    node_vectors = np.asarray(node_vectors, dtype=np.float32)

    n, dim = node_vectors.shape
    eq = edges.shape[1]
    s_nodes = np.asarray(edges[0]).astype(np.int64)
    t_nodes = np.asarray(edges[1]).astype(np.int64)

    tiles_per_core = -(-n // (NCORES * P))
    tiles_per_core += tiles_per_core % 2  # even, for pair-gathers
    shard = tiles_per_core * P
    npad = NCORES * shard
    half = npad // 2
    ntiles = NCORES * tiles_per_core
    assert half <= 32767, "table half must fit int16 gather indices"

    deg = np.bincount(adj_row, minlength=n).astype(np.float32)

    # degree-balanced relabeling: snake rows (sorted by degree desc) across
    # all tiles so each tile carries ~the same number of edges.
    order_rows = np.argsort(-deg, kind="stable")
    slot_ids = np.arange(npad)
    rounds = slot_ids // ntiles                    # 0..127 (= row slot in tile)
    pos = slot_ids % ntiles
    tiles_seq = np.where(rounds % 2 == 0, pos, ntiles - 1 - pos)
    new_ids_seq = tiles_seq * P + rounds           # new id for degree-rank r
    perm = np.full(npad, -1, np.int64)             # new_id -> old_id
    perm[new_ids_seq[:n]] = order_rows
    valid = perm >= 0
    pi = np.full(n, -1, np.int64)                  # old_id -> new_id
    pi[perm[valid]] = np.nonzero(valid)[0]

    # second pass: within each (round, table-half) the rows have ~equal total
    # degree, so permuting them across that half's tiles keeps tile totals
    # balanced while evening out each tile's lo/hi split (which otherwise
    # drifts binomially and costs a whole extra 128-slot gather chunk).
    is_lo_col0 = pi[adj_col] < half
    dlo = np.bincount(adj_row[is_lo_col0], minlength=n)
    htiles = ntiles // 2
    lo_load = np.zeros(ntiles, np.int64)
    perm2 = np.full(npad, -1, np.int64)
    for r in range(npad // ntiles):
        base = r * ntiles
        for hh in range(2):
            tset = np.arange(hh * htiles, (hh + 1) * htiles)
            slots = tset * P + r
            olds = perm[slots]
            ok = olds >= 0
            rdlo = np.where(ok, dlo[np.where(ok, olds, 0)], -1)
            row_order = np.argsort(-rdlo, kind="stable")
            tile_order = tset[np.argsort(lo_load[tset], kind="stable")]
            chosen = olds[row_order]
            dest = tile_order * P + r
            perm2[dest] = chosen
            okc = chosen >= 0
            lo_load[tile_order[okc]] += rdlo[row_order][okc]
    perm = perm2
    valid = perm >= 0
    pi = np.full(n, -1, np.int64)
    pi[perm[valid]] = np.nonzero(valid)[0]

    row_new = pi[adj_row]
    col_new = pi[adj_col]
    s_new = pi[s_nodes]
    t_new = pi[t_nodes]

    w_bf = node_weight.astype(ml_dtypes.bfloat16)
    nv_pad = np.zeros((npad, dim), ml_dtypes.bfloat16)
    nv_pad[valid] = node_vectors.astype(ml_dtypes.bfloat16)[perm[valid]]

    core_of = row_new // shard
    tile_of = (row_new % shard) // P
    rl_of = row_new % P
    is_lo = col_new < half

    key = core_of * tiles_per_core + tile_of
    cnt_lo = np.bincount(key[is_lo], minlength=ntiles)
    cnt_hi = np.bincount(key[~is_lo], minlength=ntiles)
    c_lo = max(1, int(-(-cnt_lo.max() // P)))
    c_hi = max(1, int(-(-cnt_hi.max() // P)))
    c_tot = c_lo + c_hi

    order = np.lexsort((~is_lo, tile_of, core_of))

    # ---- query groups ----
    q_core = np.repeat(np.arange(NCORES), -(-eq // NCORES))[:eq]
    q_group = np.where(s_new < half, 0, 2) + np.where(t_new < half, 0, 1)
    grp_cnt = np.zeros((NCORES, 4), np.int64)
    for k in range(NCORES):
        m = q_core == k
        grp_cnt[k] = np.bincount(q_group[m], minlength=4)
    ch = max(1, int(-(-grp_cnt.max() // P)))

    cache_key = (dim, npad, tiles_per_core, c_lo, c_hi, ch)
    if cache_key not in _CACHE:
        _CACHE[cache_key] = _build_program(dim, npad, tiles_per_core, c_lo, c_hi, ch)
    nc = _CACHE[cache_key]

    wcol_bf = w_bf[adj_col].astype(np.float32)
    deg_new = np.zeros(npad, np.float32)
    deg_new[valid] = deg[perm[valid]]
    w_new = np.zeros(npad, np.float32)
    w_new[valid] = w_bf[perm[valid]].astype(np.float32)

    in_maps = []
    q_positions = []
    for k in range(NCORES):
        sel = order[core_of[order] == k]
        idx_lo_arr = np.zeros((tiles_per_core, c_lo * P), np.int16)
        idx_hi_arr = np.zeros((tiles_per_core, c_hi * P), np.int16)
        rl_arr = np.full((P, tiles_per_core * c_tot), 255.0, np.float32)
        w_arr = np.zeros((P, tiles_per_core * c_tot), np.float32)
        for t in range(tiles_per_core):
            et = sel[tile_of[sel] == t]
            lo_e = et[is_lo[et]]
            hi_e = et[~is_lo[et]]
            nl, nh = len(lo_e), len(hi_e)
            idx_lo_arr[t, :nl] = col_new[lo_e]
            idx_hi_arr[t, :nh] = col_new[hi_e] - half
            slots = np.arange(nl)
            rl_arr[slots % P, t * c_tot + slots // P] = rl_of[lo_e]
            w_arr[slots % P, t * c_tot + slots // P] = wcol_bf[lo_e]
            slots = np.arange(nh)
            rl_arr[slots % P, t * c_tot + c_lo + slots // P] = rl_of[hi_e]
            w_arr[slots % P, t * c_tot + c_lo + slots // P] = wcol_bf[hi_e]

        idx_lo_w = np.concatenate([_wrap16(idx_lo_arr[t]) for t in range(tiles_per_core)], axis=1)
        idx_hi_w = np.concatenate([_wrap16(idx_hi_arr[t]) for t in range(tiles_per_core)], axis=1)

        qsel = np.nonzero(q_core == k)[0]
        qidx_s_arr = np.zeros((4, ch * P), np.int16)
        qidx_t_arr = np.zeros((4, ch * P), np.int16)
        qsc_arr = np.zeros((P, 4 * 4 * ch), np.float32)
        qpos = np.full((4, ch * P), -1, np.int64)
        for g in range(4):
            qg = qsel[q_group[qsel] == g]
            qg = qg[np.argsort(s_new[qg], kind="stable")]
            m = len(qg)
            sv = s_new[qg]
            tv = t_new[qg]
            qidx_s_arr[g, :m] = np.where(sv < half, sv, sv - half)
            qidx_t_arr[g, :m] = np.where(tv < half, tv, tv - half)
            qpos[g, :m] = qg
            slots = np.arange(m)
            pcol = (slots % P, slots // P)
            base = g * 4 * ch
            qsc_arr[pcol[0], base + pcol[1]] = deg_new[sv]
            qsc_arr[pcol[0], base + ch + pcol[1]] = w_new[sv]
            qsc_arr[pcol[0], base + 2 * ch + pcol[1]] = deg_new[tv]
            qsc_arr[pcol[0], base + 3 * ch + pcol[1]] = w_new[tv]

        qidx_s_w = np.concatenate([_wrap16(qidx_s_arr[g]) for g in range(4)], axis=1)
        qidx_t_w = np.concatenate([_wrap16(qidx_t_arr[g]) for g in range(4)], axis=1)

        in_maps.append({
            "nv": np.ascontiguousarray(nv_pad[k * shard:(k + 1) * shard]),
            "idx_lo": idx_lo_w,
            "idx_hi": idx_hi_w,
            "rl": rl_arr.astype(ml_dtypes.bfloat16),
            "w": w_arr.astype(ml_dtypes.bfloat16),
            "qidx_s": qidx_s_w,
            "qidx_t": qidx_t_w,
            "qsc": qsc_arr.astype(ml_dtypes.bfloat16),
        })
        q_positions.append(qpos)

    return nc, in_maps, q_positions, eq, ch


def kernel(edges, adj_row, adj_col, node_weight, node_vectors):
    _patch_cc_flags()
    nc, in_maps, q_positions, eq, ch = _prepare(
        edges, adj_row, adj_col, node_weight, node_vectors)
    res = run_bass_kernel_spmd(nc, in_maps, core_ids=list(range(NCORES)))
    outs = [res.results[k]["out"] for k in range(NCORES)]
    return _assemble(outs, q_positions, eq, ch)


def _assemble(outs, q_positions, eq, ch):
    counts = [np.zeros(eq, np.float32) for _ in range(4)]
    for k in range(NCORES):
        out = outs[k]  # [4, 128, 4*ch]
        for g in range(4):
            qpos = q_positions[k][g]
            slots = np.nonzero(qpos >= 0)[0]
            pp = slots % P
            cc = g * ch + slots // P
            for j in range(4):
                counts[j][qpos[slots]] = out[j, pp, cc]
    return tuple(counts)


# revision 21
# speedup vs baseline: 21.8305x; 21.1550x over previous
"""DotHash GNN message-passing kernel for 8 Trainium2 NeuronCores.

Strategy (1D graph/data parallel, per the sharding hint):
- Node rows are sharded 8 ways.  The host relabels nodes (degree-balanced
  snake assignment) so every 128-row tile carries a near-equal number of
  adjacency edges, and pads the node count so every core owns the same even
  number of tiles.
- node_vectors are uploaded as bf16 shards and AllGathered on device into a
  per-core table.
- Each core computes its shard of one_hop = A @ (w*nv) and two_hop =
  A @ one_hop with a matmul-based segment sum: for each pair of 128-row
  tiles, dma_gather the needed source rows (lo/hi table halves keep the
  int16 gather indices in range), build a one-hot selection matrix S on the
  vector engine (iota compare against each edge slot's local row id), and
  accumulate S.T @ G in PSUM.  node_weight is folded into S for hop one.
- one_hop / two_hop shards are exchanged with AllGather so every core holds
  the full tables.
- Query edges are sharded 8 ways and sorted into 4 groups by which table
  half their endpoints fall in; each group's rows are fetched with one
  dma_gather per table and the four dot-product families are computed with
  whole-group tensor_tensor + tensor_reduce ops (the compiler config
  forbids dynamic offsets on vector ops, so everything is static).
All floating-point math happens on device (bf16 storage, fp32 accumulate);
the host only sorts/pads/wraps integer index streams and casts dtypes.
"""

import os
import sys

import numpy as np

for _p in ("/opt/trn_rl_repo", "/root/.axon_site/_ro/trn_rl_repo"):
    if os.path.isdir(_p) and _p not in sys.path:
        sys.path.insert(0, _p)

import ml_dtypes  # noqa: E402
import concourse.bass as bass  # noqa: E402
import concourse.bacc as bacc  # noqa: E402
import concourse.mybir as mybir  # noqa: E402
import concourse.tile as tile  # noqa: E402
from concourse.bass_utils import run_bass_kernel_spmd  # noqa: E402

NCORES = 8
P = 128
bf16 = mybir.dt.bfloat16
f32 = mybir.dt.float32
i16 = mybir.dt.int16

_CACHE = {}


def _patch_cc_flags():
    """neuronxcc's DataLocalityOpt pass crashes on this program at full
    scale (assert isinstance(load.tensor, NeuronLocalTensor)); skip it."""
    from concourse import compiler_utils
    flags = compiler_utils.get_compiler_flags()
    tflag = next((f for f in flags if f.startswith("--tensorizer-options=")), None)
    if tflag is not None and "DataLocalityOpt" not in tflag:
        compiler_utils.set_compiler_flags(
            flags + [tflag + " --skip-pass=DataLocalityOpt"])


def _wrap16(idx):
    """Pack an int16 index vector (len % 128 == 0) into the [16, n/16]
    wrapped layout that dma_gather expects (idx i at [i%16, i//16])."""
    return idx.reshape(-1, 16).T.astype(np.int16)


def _build_program(dim, npad, tiles_per_core, c_lo, c_hi, ch):
    """Build the SPMD bass program.  All sizes are compile-time constants.

    ch: padded chunk count per query group (same for all groups/cores).
    """
    half = npad // 2
    shard = tiles_per_core * P
    c_tot = c_lo + c_hi
    npairs = tiles_per_core // 2

    nc = bacc.Bacc("TRN2", target_bir_lowering=False, debug=False,
                   num_devices=NCORES, num_swdge_queues=1)

    nv_in = nc.dram_tensor("nv", [shard, dim], bf16, kind="ExternalInput")
    idx_lo_d = nc.dram_tensor("idx_lo", [16, tiles_per_core * c_lo * 8], i16, kind="ExternalInput")
    idx_hi_d = nc.dram_tensor("idx_hi", [16, tiles_per_core * c_hi * 8], i16, kind="ExternalInput")
    rl_d = nc.dram_tensor("rl", [P, tiles_per_core * c_tot], bf16, kind="ExternalInput")
    w_d = nc.dram_tensor("w", [P, tiles_per_core * c_tot], bf16, kind="ExternalInput")
    qidx_s_d = nc.dram_tensor("qidx_s", [16, 4 * ch * 8], i16, kind="ExternalInput")
    qidx_t_d = nc.dram_tensor("qidx_t", [16, 4 * ch * 8], i16, kind="ExternalInput")
    qsc_d = nc.dram_tensor("qsc", [P, 4 * 4 * ch], bf16, kind="ExternalInput")
    out_d = nc.dram_tensor("out", [4, P, 4 * ch], f32, kind="ExternalOutput")

    dbg_mode = os.environ.get("KDBG", "")
    dbg_d = nc.dram_tensor("dbg", [npad, dim], bf16, kind="ExternalOutput") if dbg_mode else None

    nv_bounce = nc.dram_tensor("nv_bounce", [shard, dim], bf16)
    shared_as = "Shared" if os.environ.get("KSHARED", "0") == "1" else "Local"
    nv_table = nc.dram_tensor("nv_table", [npad, dim], bf16, addr_space=shared_as)
    oh_bounce = nc.dram_tensor("oh_bounce", [shard, dim], bf16)
    oh_table = nc.dram_tensor("oh_table", [npad, dim], bf16, addr_space=shared_as)
    comb_bounce = nc.dram_tensor("comb_bounce", [shard, 2 * dim], bf16)
    comb_table = nc.dram_tensor("comb_table", [npad, 2 * dim], bf16, addr_space=shared_as)

    krep = int(os.environ.get("KREP", "1"))

    # idx arrays arrive as [16, X] (the dma_gather wrap layout); the Q7
    # ucode wants them replicated across all 128 partitions, so expand them
    # once into internal DRAM with a broadcast DMA, then load slices.
    idx_reps = {}
    for nm, src_t in (("idx_lo", idx_lo_d), ("idx_hi", idx_hi_d),
                      ("qidx_s", qidx_s_d), ("qidx_t", qidx_t_d)):
        xcols = src_t.shape[1]
        rep_t = nc.dram_tensor(f"{nm}_rep", [P, xcols], i16)
        idx_reps[nm] = rep_t

    def replicate_idx():
        for nm, src_t in (("idx_lo", idx_lo_d), ("idx_hi", idx_hi_d),
                          ("qidx_s", qidx_s_d), ("qidx_t", qidx_t_d)):
            xcols = src_t.shape[1]
            rep_t = idx_reps[nm]
            sap = src_t[:]
            rep_src = bass.AP(sap.tensor, sap.offset,
                              [[0, 8], list(sap.ap[0]), list(sap.ap[1])])
            nc.sync.dma_start(rep_t[:].rearrange("(a b) c -> a b c", a=8), rep_src)

    def load_idx(pool, tag, src_rep, col0, ncols):
        t = pool.tile([P, ncols], i16, tag=tag, name=tag)
        nc.sync.dma_start(t[:], src_rep[:, bass.ds(col0, ncols)])
        return t

    # KGMAX<999 splits each gather into single-packet <=1024-idx pieces;
    # whole-kernel A/B measured the unsplit version faster, so default off.
    GMAX = int(os.environ.get("KGMAX", "999"))

    def split_gather(gt, tab, idxt, nchunks, elem):
        u0 = 0
        while u0 < nchunks:
            nch = min(GMAX, nchunks - u0)
            nc.gpsimd.dma_gather(
                gt[:, u0:u0 + nch, :], tab, idxt[:, u0 * 8:(u0 + nch) * 8],
                nch * P, nch * P, elem,
                single_packet=nch * P <= 1024, queue_num=0)
            u0 += nch

    def spmm_phase(tc, table_lo, table_hi, shard_sb, weighted, iota_t,
                   meta_pool, g_pool, s_pool, psum_pool, krep_phase=None):
        def body(i):
            idx_lo = load_idx(meta_pool, "idxlo", idx_reps["idx_lo"], i * (2 * c_lo * 8), 2 * c_lo * 8)
            idx_hi = load_idx(meta_pool, "idxhi", idx_reps["idx_hi"], i * (2 * c_hi * 8), 2 * c_hi * 8)
            rl_t = meta_pool.tile([P, 2 * c_tot], bf16, tag="rl")
            nc.sync.dma_start(rl_t[:], rl_d[:, bass.ds(i * 2 * c_tot, 2 * c_tot)])

            g_lo = g_pool.tile([P, 2 * c_lo, dim], bf16, tag="glo", name="g_lo")
            g_hi = g_pool.tile([P, 2 * c_hi, dim], bf16, tag="ghi", name="g_hi")
            for gt, tab, idxt, cc2 in ((g_lo, table_lo, idx_lo, 2 * c_lo),
                                       (g_hi, table_hi, idx_hi, 2 * c_hi)):
                split_gather(gt, tab, idxt, cc2, dim)

            s = s_pool.tile([P, 2 * c_tot * P], bf16, tag="s")
            rl_ap = rl_t[:]
            nc.vector.tensor_tensor(
                out=s[:],
                in0=bass.AP(rl_ap.tensor, rl_ap.offset,
                            [rl_ap.ap[0], [1, 2 * c_tot], [0, P]]),
                in1=iota_t[:].rearrange("p (c m) -> p c m", c=2 * c_tot),
                op=mybir.AluOpType.is_equal)
            if weighted:
                w_t = meta_pool.tile([P, 2 * c_tot], bf16, tag="w")
                nc.sync.dma_start(w_t[:], w_d[:, bass.ds(i * 2 * c_tot, 2 * c_tot)])
                w_ap = w_t[:]
                nc.vector.tensor_tensor(
                    out=s[:],
                    in0=s[:].rearrange("p (c m) -> p c m", c=2 * c_tot),
                    in1=bass.AP(w_ap.tensor, w_ap.offset,
                                [w_ap.ap[0], [1, 2 * c_tot], [0, P]]),
                    op=mybir.AluOpType.mult)

            # pair-half h (tile 2i+h) uses S chunks h*c_tot + cc; its lo
            # chunks sit at g_lo[:, h*c_lo + cc], hi at g_hi[:, h*c_hi + ...].
            for h in range(2):
                ps = psum_pool.tile([P, dim], f32, tag="ps")
                for cc in range(c_tot):
                    if cc < c_lo:
                        g_ap = g_lo[:, h * c_lo + cc, :]
                    else:
                        g_ap = g_hi[:, h * c_hi + (cc - c_lo), :]
                    sc = (h * c_tot + cc) * P
                    nc.tensor.matmul(ps[:], s[:, sc:sc + P], g_ap,
                                     start=(cc == 0), stop=(cc == c_tot - 1))
                nc.scalar.copy(shard_sb[:, bass.ds(2 * i + h, 1), :], ps[:, None, :])

        for _ in range(krep_phase if krep_phase is not None else krep):
            tc.For_i_unrolled(0, npairs, 1, body, max_unroll=2)

    with tile.TileContext(nc) as tc:
        with (
            tc.tile_pool(name="const", bufs=1) as const_pool,
        ):
            iota_t = const_pool.tile([P, 2 * c_tot * P], bf16)
            nc.gpsimd.iota(iota_t[:], pattern=[[0, 2 * c_tot], [1, P]], base=0,
                           channel_multiplier=0, allow_small_or_imprecise_dtypes=True)

            # ---- phase 0: replicate idx arrays, distribute node vectors ----
            replicate_idx()
            nc.sync.dma_start(nv_bounce[:], nv_in[:])
            nc.gpsimd.collective_compute(
                "AllGather", mybir.AluOpType.bypass,
                replica_groups=[list(range(NCORES))],
                ins=[nv_bounce[:]], outs=[nv_table[:]])

            # ---- phase A: one_hop shard ----
            with (
                tc.tile_pool(name="shardA", bufs=1) as shard_pool,
                tc.tile_pool(name="metaA", bufs=int(os.environ.get("KMBUFS", "3"))) as meta_pool,
                tc.tile_pool(name="gA", bufs=int(os.environ.get("KGBUFS", "2"))) as g_pool,
                tc.tile_pool(name="sA", bufs=2) as s_pool,
                tc.tile_pool(name="psA", bufs=2, space="PSUM") as psum_pool,
            ):
                oh_sb = shard_pool.tile([P, tiles_per_core, dim], bf16)
                spmm_phase(tc, nv_table[0:half, :], nv_table[half:npad, :], oh_sb, True,
                           iota_t, meta_pool, g_pool, s_pool, psum_pool,
                           krep_phase=int(os.environ.get("KREPA", "0")) or None)
                nc.sync.dma_start(oh_bounce[:].rearrange("(t p) d -> p t d", p=P), oh_sb[:])
            if dbg_mode == "A":
                nc.sync.dma_start(dbg_d[0:shard, :], oh_bounce[:])
            if dbg_mode != "A":
                nc.gpsimd.collective_compute(
                    "AllGather", mybir.AluOpType.bypass,
                    replica_groups=[list(range(NCORES))],
                    ins=[oh_bounce[:]], outs=[oh_table[:]])
                if dbg_mode == "AG":
                    nc.sync.dma_start(dbg_d[:], oh_table[:])

            # ---- phase B: two_hop shard ----
            if dbg_mode not in ("A", "AG"):
                with (
                    tc.tile_pool(name="shardB", bufs=1) as shard_pool,
                    tc.tile_pool(name="metaB", bufs=int(os.environ.get("KMBUFS", "3"))) as meta_pool,
                    tc.tile_pool(name="gB", bufs=int(os.environ.get("KGBUFS", "2"))) as g_pool,
                    tc.tile_pool(name="sB", bufs=2) as s_pool,
                    tc.tile_pool(name="psB", bufs=2, space="PSUM") as psum_pool,
                ):
                    th_sb = shard_pool.tile([P, tiles_per_core, dim], bf16)
                    spmm_phase(tc, oh_table[0:half, :], oh_table[half:npad, :], th_sb, False,
                               iota_t, meta_pool, g_pool, s_pool, psum_pool,
                               krep_phase=int(os.environ.get("KREPB", "0")) or None)
                    # interleave [oh | th] per row so the query phase can fetch
                    # both with one 1KB-row gather (cheaper than 2x512B).
                    nc.sync.dma_start(
                        comb_bounce[:, dim:2 * dim].rearrange("(t p) d -> p t d", p=P),
                        th_sb[:])
                nc.sync.dma_start(comb_bounce[:, 0:dim], oh_bounce[:])
                nc.gpsimd.collective_compute(
                    "AllGather", mybir.AluOpType.bypass,
                    replica_groups=[list(range(NCORES))],
                    ins=[comb_bounce[:]], outs=[comb_table[:]])
                if dbg_mode == "AB":
                    nc.sync.dma_start(dbg_d[:], comb_table[:, dim:2 * dim])

            # ---- phase C: query dots (no loops; whole-group tensors) ----
            if dbg_mode == "":
                with (
                    tc.tile_pool(name="qidx", bufs=2) as qidx_pool,
                    tc.tile_pool(name="qg", bufs=1) as qg_pool,
                    tc.tile_pool(name="qtmp", bufs=1) as qtmp_pool,
                    tc.tile_pool(name="qout", bufs=1) as qout_pool,
                ):
                    mul = mybir.AluOpType.mult
                    add = mybir.AluOpType.add
                    sub = mybir.AluOpType.subtract
                    X = mybir.AxisListType.X
                    nidx = ch * P
                    for _ in range(int(os.environ.get("KREPC", "0")) or krep):
                        for g in range(4):
                            s_lo = (g // 2) == 0
                            t_lo = (g % 2) == 0

                            def tab(t_, lo):
                                return t_[0:half, :] if lo else t_[half:npad, :]

                            idx_s = load_idx(qidx_pool, "qis", idx_reps["qidx_s"], g * ch * 8, ch * 8)
                            idx_t = load_idx(qidx_pool, "qit", idx_reps["qidx_t"], g * ch * 8, ch * 8)
                            sc_b = qidx_pool.tile([P, 4, ch], bf16, tag="scb", name="sc_b")
                            nc.sync.dma_start(sc_b[:], qsc_d[:, g * 4 * ch:(g + 1) * 4 * ch]
                                              .rearrange("p (j c) -> p j c", j=4))
                            cs_b = qidx_pool.tile([P, ch], f32, tag="csb", name="cs_b")
                            ct_b = qidx_pool.tile([P, ch], f32, tag="ctb", name="ct_b")
                            nc.vector.tensor_tensor(out=cs_b[:], in0=sc_b[:, 0, :],
                                                    in1=sc_b[:, 1, :], op=mul)
                            nc.vector.tensor_tensor(out=ct_b[:], in0=sc_b[:, 2, :],
                                                    in1=sc_b[:, 3, :], op=mul)

                            tiles = {}
                            for name, table, idxt in (
                                    ("cs", tab(comb_table, s_lo), idx_s),
                                    ("ct", tab(comb_table, t_lo), idx_t)):
                                t_ = qg_pool.tile([P, ch, 2 * dim], bf16, tag=name, name=name)
                                split_gather(t_, table, idxt, ch, 2 * dim)
                                tiles[name] = t_
                            for name, table, idxt in (
                                    ("nvs", tab(nv_table, s_lo), idx_s),
                                    ("nvt", tab(nv_table, t_lo), idx_t)):
                                t_ = qg_pool.tile([P, ch, dim], bf16, tag=name, name=name)
                                split_gather(t_, table, idxt, ch, dim)
                                tiles[name] = t_

                            acc = qout_pool.tile([P, 6, ch], f32, tag="acc", name="acc")
                            prod = qtmp_pool.tile([P, ch, dim], bf16, tag="prod", name="prod")
                            zs_t = qtmp_pool.tile([P, ch, dim], bf16, tag="zs", name="zs_t")
                            zt_t = qtmp_pool.tile([P, ch, dim], bf16, tag="zt", name="zt_t")

                            def bcast(t2d):
                                ap = t2d[:]
                                return bass.AP(ap.tensor, ap.offset,
                                               [ap.ap[0], [1, ch], [0, dim]])

                            def dot(dst_j, a_ap, b_ap):
                                nc.vector.tensor_tensor(out=prod[:], in0=a_ap, in1=b_ap, op=mul)
                                nc.vector.tensor_reduce(out=acc[:, dst_j, :], in_=prod[:],
                                                        axis=X, op=add)

                            ohs = tiles["cs"][:, :, 0:dim]
                            ths = tiles["cs"][:, :, dim:2 * dim]
                            oht = tiles["ct"][:, :, 0:dim]
                            tht = tiles["ct"][:, :, dim:2 * dim]
                            dot(0, ohs, oht)
                            dot(1, ohs, tht)
                            dot(2, ths, oht)
                            dot(4, ohs, ths)
                            dot(5, oht, tht)
                            # z = th - (deg*w) * nv
                            nc.vector.tensor_tensor(out=zs_t[:], in0=tiles["nvs"][:],
                                                    in1=bcast(cs_b), op=mul)
                            nc.vector.tensor_tensor(out=zs_t[:], in0=ths, in1=zs_t[:], op=sub)
                            nc.vector.tensor_tensor(out=zt_t[:], in0=tiles["nvt"][:],
                                                    in1=bcast(ct_b), op=mul)
                            nc.vector.tensor_tensor(out=zt_t[:], in0=tht, in1=zt_t[:], op=sub)
                            dot(3, zs_t[:], zt_t[:])
                            # c12 = acc1+acc2, cself = acc4+acc5
                            nc.vector.tensor_tensor(out=acc[:, 1, :], in0=acc[:, 1, :],
                                                    in1=acc[:, 2, :], op=add)
                            nc.vector.tensor_tensor(out=acc[:, 4, :], in0=acc[:, 4, :],
                                                    in1=acc[:, 5, :], op=add)
                            for jj, aj in enumerate((0, 1, 3, 4)):
                                nc.sync.dma_start(out_d[jj][:, g * ch:(g + 1) * ch],
                                                  acc[:, aj, :])

    nc.compile()
    return nc


def _prepare(edges, adj_row, adj_col, node_weight, node_vectors):
    edges = np.asarray(edges)
    adj_row = np.asarray(adj_row).astype(np.int64)
    adj_col = np.asarray(adj_col).astype(np.int64)
    node_weight = np.asarray(node_weight, dtype=np.float32)
    node_vectors = np.asarray(node_vectors, dtype=np.float32)

    n, dim = node_vectors.shape
    eq = edges.shape[1]
    s_nodes = np.asarray(edges[0]).astype(np.int64)
    t_nodes = np.asarray(edges[1]).astype(np.int64)

    tiles_per_core = -(-n // (NCORES * P))
    tiles_per_core += tiles_per_core % 2  # even, for pair-gathers
    shard = tiles_per_core * P
    npad = NCORES * shard
    half = npad // 2
    ntiles = NCORES * tiles_per_core
    assert half <= 32767, "table half must fit int16 gather indices"

    deg = np.bincount(adj_row, minlength=n).astype(np.float32)

    # degree-balanced relabeling: snake rows (sorted by degree desc) across
    # all tiles so each tile carries ~the same number of edges.
    order_rows = np.argsort(-deg, kind="stable")
    slot_ids = np.arange(npad)
    rounds = slot_ids // ntiles                    # 0..127 (= row slot in tile)
    pos = slot_ids % ntiles
    tiles_seq = np.where(rounds % 2 == 0, pos, ntiles - 1 - pos)
    new_ids_seq = tiles_seq * P + rounds           # new id for degree-rank r
    perm = np.full(npad, -1, np.int64)             # new_id -> old_id
    perm[new_ids_seq[:n]] = order_rows
    valid = perm >= 0
    pi = np.full(n, -1, np.int64)                  # old_id -> new_id
    pi[perm[valid]] = np.nonzero(valid)[0]

    # second pass: within each (round, table-half) the rows have ~equal total
    # degree, so permuting them across that half's tiles keeps tile totals
    # balanced while evening out each tile's lo/hi split (which otherwise
    # drifts binomially and costs a whole extra 128-slot gather chunk).
    is_lo_col0 = pi[adj_col] < half
    dlo = np.bincount(adj_row[is_lo_col0], minlength=n)
    htiles = ntiles // 2
    lo_load = np.zeros(ntiles, np.int64)
    perm2 = np.full(npad, -1, np.int64)
    for r in range(npad // ntiles):
        base = r * ntiles
        for hh in range(2):
            tset = np.arange(hh * htiles, (hh + 1) * htiles)
            slots = tset * P + r
            olds = perm[slots]
            ok = olds >= 0
            rdlo = np.where(ok, dlo[np.where(ok, olds, 0)], -1)
            row_order = np.argsort(-rdlo, kind="stable")
            tile_order = tset[np.argsort(lo_load[tset], kind="stable")]
            chosen = olds[row_order]
            dest = tile_order * P + r
            perm2[dest] = chosen
            okc = chosen >= 0
            lo_load[tile_order[okc]] += rdlo[row_order][okc]
    perm = perm2
    valid = perm >= 0
    pi = np.full(n, -1, np.int64)
    pi[perm[valid]] = np.nonzero(valid)[0]

    row_new = pi[adj_row]
    col_new = pi[adj_col]
    s_new = pi[s_nodes]
    t_new = pi[t_nodes]

    w_bf = node_weight.astype(ml_dtypes.bfloat16)
    nv_pad = np.zeros((npad, dim), ml_dtypes.bfloat16)
    nv_pad[valid] = node_vectors.astype(ml_dtypes.bfloat16)[perm[valid]]

    core_of = row_new // shard
    tile_of = (row_new % shard) // P
    rl_of = row_new % P
    is_lo = col_new < half

    key = core_of * tiles_per_core + tile_of
    cnt_lo = np.bincount(key[is_lo], minlength=ntiles)
    cnt_hi = np.bincount(key[~is_lo], minlength=ntiles)
    c_lo = max(1, int(-(-cnt_lo.max() // P)))
    c_hi = max(1, int(-(-cnt_hi.max() // P)))
    c_tot = c_lo + c_hi

    order = np.lexsort((~is_lo, tile_of, core_of))

    # ---- query groups ----
    q_core = np.repeat(np.arange(NCORES), -(-eq // NCORES))[:eq]
    q_group = np.where(s_new < half, 0, 2) + np.where(t_new < half, 0, 1)
    grp_cnt = np.zeros((NCORES, 4), np.int64)
    for k in range(NCORES):
        m = q_core == k
        grp_cnt[k] = np.bincount(q_group[m], minlength=4)
    ch = max(1, int(-(-grp_cnt.max() // P)))

    cache_key = (dim, npad, tiles_per_core, c_lo, c_hi, ch)
    if cache_key not in _CACHE:
        _CACHE[cache_key] = _build_program(dim, npad, tiles_per_core, c_lo, c_hi, ch)
    nc = _CACHE[cache_key]

    wcol_bf = w_bf[adj_col].astype(np.float32)
    deg_new = np.zeros(npad, np.float32)
    deg_new[valid] = deg[perm[valid]]
    w_new = np.zeros(npad, np.float32)
    w_new[valid] = w_bf[perm[valid]].astype(np.float32)

    in_maps = []
    q_positions = []
    for k in range(NCORES):
        sel = order[core_of[order] == k]
        idx_lo_arr = np.zeros((tiles_per_core, c_lo * P), np.int16)
        idx_hi_arr = np.zeros((tiles_per_core, c_hi * P), np.int16)
        rl_arr = np.full((P, tiles_per_core * c_tot), 255.0, np.float32)
        w_arr = np.zeros((P, tiles_per_core * c_tot), np.float32)
        for t in range(tiles_per_core):
            et = sel[tile_of[sel] == t]
            lo_e = et[is_lo[et]]
            hi_e = et[~is_lo[et]]
            nl, nh = len(lo_e), len(hi_e)
            idx_lo_arr[t, :nl] = col_new[lo_e]
            idx_hi_arr[t, :nh] = col_new[hi_e] - half
            slots = np.arange(nl)
            rl_arr[slots % P, t * c_tot + slots // P] = rl_of[lo_e]
            w_arr[slots % P, t * c_tot + slots // P] = wcol_bf[lo_e]
            slots = np.arange(nh)
            rl_arr[slots % P, t * c_tot + c_lo + slots // P] = rl_of[hi_e]
            w_arr[slots % P, t * c_tot + c_lo + slots // P] = wcol_bf[hi_e]

        idx_lo_w = np.concatenate([_wrap16(idx_lo_arr[t]) for t in range(tiles_per_core)], axis=1)
        idx_hi_w = np.concatenate([_wrap16(idx_hi_arr[t]) for t in range(tiles_per_core)], axis=1)

        qsel = np.nonzero(q_core == k)[0]
        qidx_s_arr = np.zeros((4, ch * P), np.int16)
        qidx_t_arr = np.zeros((4, ch * P), np.int16)
        qsc_arr = np.zeros((P, 4 * 4 * ch), np.float32)
        qpos = np.full((4, ch * P), -1, np.int64)
        for g in range(4):
            qg = qsel[q_group[qsel] == g]
            qg = qg[np.argsort(s_new[qg], kind="stable")]
            m = len(qg)
            sv = s_new[qg]
            tv = t_new[qg]
            qidx_s_arr[g, :m] = np.where(sv < half, sv, sv - half)
            qidx_t_arr[g, :m] = np.where(tv < half, tv, tv - half)
            qpos[g, :m] = qg
            slots = np.arange(m)
            pcol = (slots % P, slots // P)
            base = g * 4 * ch
            qsc_arr[pcol[0], base + pcol[1]] = deg_new[sv]
            qsc_arr[pcol[0], base + ch + pcol[1]] = w_new[sv]
            qsc_arr[pcol[0], base + 2 * ch + pcol[1]] = deg_new[tv]
            qsc_arr[pcol[0], base + 3 * ch + pcol[1]] = w_new[tv]

        qidx_s_w = np.concatenate([_wrap16(qidx_s_arr[g]) for g in range(4)], axis=1)
        qidx_t_w = np.concatenate([_wrap16(qidx_t_arr[g]) for g in range(4)], axis=1)

        in_maps.append({
            "nv": np.ascontiguousarray(nv_pad[k * shard:(k + 1) * shard]),
            "idx_lo": idx_lo_w,
            "idx_hi": idx_hi_w,
            "rl": rl_arr.astype(ml_dtypes.bfloat16),
            "w": w_arr.astype(ml_dtypes.bfloat16),
            "qidx_s": qidx_s_w,
            "qidx_t": qidx_t_w,
            "qsc": qsc_arr.astype(ml_dtypes.bfloat16),
        })
        q_positions.append(qpos)

    return nc, in_maps, q_positions, eq, ch


def kernel(edges, adj_row, adj_col, node_weight, node_vectors):
    _patch_cc_flags()
    nc, in_maps, q_positions, eq, ch = _prepare(
        edges, adj_row, adj_col, node_weight, node_vectors)
    res = run_bass_kernel_spmd(nc, in_maps, core_ids=list(range(NCORES)))
    outs = [res.results[k]["out"] for k in range(NCORES)]
    return _assemble(outs, q_positions, eq, ch)


def _assemble(outs, q_positions, eq, ch):
    counts = [np.zeros(eq, np.float32) for _ in range(4)]
    for k in range(NCORES):
        out = outs[k]  # [4, 128, 4*ch]
        for g in range(4):
            qpos = q_positions[k][g]
            slots = np.nonzero(qpos >= 0)[0]
            pp = slots % P
            cc = g * ch + slots // P
            for j in range(4):
                counts[j][qpos[slots]] = out[j, pp, cc]
    return tuple(counts)


# revision 23
# speedup vs baseline: 50.8087x; 2.3274x over previous
"""DotHash GNN message-passing kernel for 8 Trainium2 NeuronCores.

Strategy (1D graph/data parallel, per the sharding hint):
- Node rows are sharded 8 ways.  The host relabels nodes (degree-balanced
  snake assignment) so every 128-row tile carries a near-equal number of
  adjacency edges, and pads the node count so every core owns the same even
  number of tiles.
- node_vectors are uploaded as bf16 shards and AllGathered on device into a
  per-core table.
- Each core computes its shard of one_hop = A @ (w*nv) and two_hop =
  A @ one_hop with a matmul-based segment sum: for each pair of 128-row
  tiles, dma_gather the needed source rows (lo/hi table halves keep the
  int16 gather indices in range), build a one-hot selection matrix S on the
  vector engine (iota compare against each edge slot's local row id), and
  accumulate S.T @ G in PSUM.  node_weight is folded into S for hop one.
- one_hop / two_hop shards are exchanged with AllGather so every core holds
  the full tables.
- Query edges are sharded 8 ways and sorted into 4 groups by which table
  half their endpoints fall in; each group's rows are fetched with one
  dma_gather per table and the four dot-product families are computed with
  whole-group tensor_tensor + tensor_reduce ops (the compiler config
  forbids dynamic offsets on vector ops, so everything is static).
All floating-point math happens on device (bf16 storage, fp32 accumulate);
the host only sorts/pads/wraps integer index streams and casts dtypes.
"""

import os
import sys

import numpy as np

for _p in ("/opt/trn_rl_repo", "/root/.axon_site/_ro/trn_rl_repo"):
    if os.path.isdir(_p) and _p not in sys.path:
        sys.path.insert(0, _p)

import ml_dtypes  # noqa: E402
import concourse.bass as bass  # noqa: E402
import concourse.bacc as bacc  # noqa: E402
import concourse.mybir as mybir  # noqa: E402
import concourse.tile as tile  # noqa: E402
from concourse.bass_utils import run_bass_kernel_spmd  # noqa: E402

NCORES = 8
P = 128
bf16 = mybir.dt.bfloat16
f32 = mybir.dt.float32
i16 = mybir.dt.int16

_CACHE = {}


def _patch_cc_flags():
    """neuronxcc's DataLocalityOpt pass crashes on this program at full
    scale (assert isinstance(load.tensor, NeuronLocalTensor)); skip it."""
    from concourse import compiler_utils
    flags = compiler_utils.get_compiler_flags()
    tflag = next((f for f in flags if f.startswith("--tensorizer-options=")), None)
    if tflag is not None and "DataLocalityOpt" not in tflag:
        compiler_utils.set_compiler_flags(
            flags + [tflag + " --skip-pass=DataLocalityOpt"])


def _wrap16(idx):
    """Pack an int16 index vector (len % 128 == 0) into the [16, n/16]
    wrapped layout that dma_gather expects (idx i at [i%16, i//16])."""
    return idx.reshape(-1, 16).T.astype(np.int16)


def _build_program(dim, npad, tiles_per_core, c_lo, c_hi, ch):
    """Build the SPMD bass program.  All sizes are compile-time constants.

    ch: padded chunk count per query group (same for all groups/cores).
    """
    half = npad // 2
    shard = tiles_per_core * P
    c_tot = c_lo + c_hi
    npairs = tiles_per_core // 2

    nc = bacc.Bacc("TRN2", target_bir_lowering=False, debug=False,
                   num_devices=NCORES, num_swdge_queues=1)

    nv_in = nc.dram_tensor("nv", [shard, dim], bf16, kind="ExternalInput")
    idx_lo_d = nc.dram_tensor("idx_lo", [16, tiles_per_core * c_lo * 8], i16, kind="ExternalInput")
    idx_hi_d = nc.dram_tensor("idx_hi", [16, tiles_per_core * c_hi * 8], i16, kind="ExternalInput")
    rl_d = nc.dram_tensor("rl", [P, tiles_per_core * c_tot], bf16, kind="ExternalInput")
    w_d = nc.dram_tensor("w", [P, tiles_per_core * c_tot], bf16, kind="ExternalInput")
    qidx_s_d = nc.dram_tensor("qidx_s", [16, 4 * ch * 8], i16, kind="ExternalInput")
    qidx_t_d = nc.dram_tensor("qidx_t", [16, 4 * ch * 8], i16, kind="ExternalInput")
    qsc_d = nc.dram_tensor("qsc", [P, 4 * 4 * ch], bf16, kind="ExternalInput")
    out_d = nc.dram_tensor("out", [4, P, 4 * ch], f32, kind="ExternalOutput")

    dbg_mode = os.environ.get("KDBG", "")
    dbg_d = nc.dram_tensor("dbg", [npad, dim], bf16, kind="ExternalOutput") if dbg_mode else None

    nv_bounce = nc.dram_tensor("nv_bounce", [shard, dim], bf16)
    shared_as = "Shared" if os.environ.get("KSHARED", "0") == "1" else "Local"
    nv_table = nc.dram_tensor("nv_table", [npad, dim], bf16, addr_space=shared_as)
    oh_bounce = nc.dram_tensor("oh_bounce", [shard, dim], bf16)
    oh_table = nc.dram_tensor("oh_table", [npad, dim], bf16, addr_space=shared_as)
    comb_bounce = nc.dram_tensor("comb_bounce", [shard, 3 * dim], bf16)
    comb_table = nc.dram_tensor("comb_table", [npad, 3 * dim], bf16, addr_space=shared_as)

    krep = int(os.environ.get("KREP", "1"))

    # idx arrays arrive as [16, X] (the dma_gather wrap layout); the Q7
    # ucode wants them replicated across all 128 partitions, so expand them
    # once into internal DRAM with a broadcast DMA, then load slices.
    idx_reps = {}
    for nm, src_t in (("idx_lo", idx_lo_d), ("idx_hi", idx_hi_d),
                      ("qidx_s", qidx_s_d), ("qidx_t", qidx_t_d)):
        xcols = src_t.shape[1]
        rep_t = nc.dram_tensor(f"{nm}_rep", [P, xcols], i16)
        idx_reps[nm] = rep_t

    def replicate_idx():
        for nm, src_t in (("idx_lo", idx_lo_d), ("idx_hi", idx_hi_d),
                          ("qidx_s", qidx_s_d), ("qidx_t", qidx_t_d)):
            xcols = src_t.shape[1]
            rep_t = idx_reps[nm]
            sap = src_t[:]
            rep_src = bass.AP(sap.tensor, sap.offset,
                              [[0, 8], list(sap.ap[0]), list(sap.ap[1])])
            nc.sync.dma_start(rep_t[:].rearrange("(a b) c -> a b c", a=8), rep_src)

    def load_idx(pool, tag, src_rep, col0, ncols):
        t = pool.tile([P, ncols], i16, tag=tag, name=tag)
        nc.sync.dma_start(t[:], src_rep[:, bass.ds(col0, ncols)])
        return t

    # Split each gather into single-packet <=1024-idx pieces: with the
    # 1536B-row combined query table this measured 30.8 vs 40.1 ms/iter
    # against one big multi-packet gather.
    GMAX = int(os.environ.get("KGMAX", "8"))

    def split_gather(gt, tab, idxt, nchunks, elem):
        u0 = 0
        while u0 < nchunks:
            nch = min(GMAX, nchunks - u0)
            nc.gpsimd.dma_gather(
                gt[:, u0:u0 + nch, :], tab, idxt[:, u0 * 8:(u0 + nch) * 8],
                nch * P, nch * P, elem,
                single_packet=nch * P <= 1024, queue_num=0)
            u0 += nch

    def spmm_phase(tc, table_lo, table_hi, shard_sb, weighted, iota_t,
                   meta_pool, g_pool, s_pool, psum_pool, krep_phase=None):
        def body(i):
            idx_lo = load_idx(meta_pool, "idxlo", idx_reps["idx_lo"], i * (2 * c_lo * 8), 2 * c_lo * 8)
            idx_hi = load_idx(meta_pool, "idxhi", idx_reps["idx_hi"], i * (2 * c_hi * 8), 2 * c_hi * 8)
            rl_t = meta_pool.tile([P, 2 * c_tot], bf16, tag="rl")
            nc.sync.dma_start(rl_t[:], rl_d[:, bass.ds(i * 2 * c_tot, 2 * c_tot)])

            g_lo = g_pool.tile([P, 2 * c_lo, dim], bf16, tag="glo", name="g_lo")
            g_hi = g_pool.tile([P, 2 * c_hi, dim], bf16, tag="ghi", name="g_hi")
            for gt, tab, idxt, cc2 in ((g_lo, table_lo, idx_lo, 2 * c_lo),
                                       (g_hi, table_hi, idx_hi, 2 * c_hi)):
                split_gather(gt, tab, idxt, cc2, dim)

            s = s_pool.tile([P, 2 * c_tot * P], bf16, tag="s")
            rl_ap = rl_t[:]
            nc.vector.tensor_tensor(
                out=s[:],
                in0=bass.AP(rl_ap.tensor, rl_ap.offset,
                            [rl_ap.ap[0], [1, 2 * c_tot], [0, P]]),
                in1=iota_t[:].rearrange("p (c m) -> p c m", c=2 * c_tot),
                op=mybir.AluOpType.is_equal)
            if weighted:
                w_t = meta_pool.tile([P, 2 * c_tot], bf16, tag="w")
                nc.sync.dma_start(w_t[:], w_d[:, bass.ds(i * 2 * c_tot, 2 * c_tot)])
                w_ap = w_t[:]
                nc.vector.tensor_tensor(
                    out=s[:],
                    in0=s[:].rearrange("p (c m) -> p c m", c=2 * c_tot),
                    in1=bass.AP(w_ap.tensor, w_ap.offset,
                                [w_ap.ap[0], [1, 2 * c_tot], [0, P]]),
                    op=mybir.AluOpType.mult)

            # pair-half h (tile 2i+h) uses S chunks h*c_tot + cc; its lo
            # chunks sit at g_lo[:, h*c_lo + cc], hi at g_hi[:, h*c_hi + ...].
            for h in range(2):
                ps = psum_pool.tile([P, dim], f32, tag="ps")
                for cc in range(c_tot):
                    if cc < c_lo:
                        g_ap = g_lo[:, h * c_lo + cc, :]
                    else:
                        g_ap = g_hi[:, h * c_hi + (cc - c_lo), :]
                    sc = (h * c_tot + cc) * P
                    nc.tensor.matmul(ps[:], s[:, sc:sc + P], g_ap,
                                     start=(cc == 0), stop=(cc == c_tot - 1))
                nc.scalar.copy(shard_sb[:, bass.ds(2 * i + h, 1), :], ps[:, None, :])

        for _ in range(krep_phase if krep_phase is not None else krep):
            tc.For_i_unrolled(0, npairs, 1, body, max_unroll=2)

    with tile.TileContext(nc) as tc:
        with (
            tc.tile_pool(name="const", bufs=1) as const_pool,
        ):
            iota_t = const_pool.tile([P, 2 * c_tot * P], bf16)
            nc.gpsimd.iota(iota_t[:], pattern=[[0, 2 * c_tot], [1, P]], base=0,
                           channel_multiplier=0, allow_small_or_imprecise_dtypes=True)

            # ---- phase 0: replicate idx arrays, distribute node vectors ----
            replicate_idx()
            nc.sync.dma_start(nv_bounce[:], nv_in[:])
            nc.gpsimd.collective_compute(
                "AllGather", mybir.AluOpType.bypass,
                replica_groups=[list(range(NCORES))],
                ins=[nv_bounce[:]], outs=[nv_table[:]])

            # ---- phase A: one_hop shard ----
            with (
                tc.tile_pool(name="shardA", bufs=1) as shard_pool,
                tc.tile_pool(name="metaA", bufs=int(os.environ.get("KMBUFS", "3"))) as meta_pool,
                tc.tile_pool(name="gA", bufs=int(os.environ.get("KGBUFS", "2"))) as g_pool,
                tc.tile_pool(name="sA", bufs=2) as s_pool,
                tc.tile_pool(name="psA", bufs=2, space="PSUM") as psum_pool,
            ):
                oh_sb = shard_pool.tile([P, tiles_per_core, dim], bf16)
                spmm_phase(tc, nv_table[0:half, :], nv_table[half:npad, :], oh_sb, True,
                           iota_t, meta_pool, g_pool, s_pool, psum_pool,
                           krep_phase=int(os.environ.get("KREPA", "0")) or None)
                nc.sync.dma_start(oh_bounce[:].rearrange("(t p) d -> p t d", p=P), oh_sb[:])
            if dbg_mode == "A":
                nc.sync.dma_start(dbg_d[0:shard, :], oh_bounce[:])
            if dbg_mode != "A":
                nc.gpsimd.collective_compute(
                    "AllGather", mybir.AluOpType.bypass,
                    replica_groups=[list(range(NCORES))],
                    ins=[oh_bounce[:]], outs=[oh_table[:]])
                if dbg_mode == "AG":
                    nc.sync.dma_start(dbg_d[:], oh_table[:])

            # ---- phase B: two_hop shard ----
            if dbg_mode not in ("A", "AG"):
                with (
                    tc.tile_pool(name="shardB", bufs=1) as shard_pool,
                    tc.tile_pool(name="metaB", bufs=int(os.environ.get("KMBUFS", "3"))) as meta_pool,
                    tc.tile_pool(name="gB", bufs=int(os.environ.get("KGBUFS", "2"))) as g_pool,
                    tc.tile_pool(name="sB", bufs=2) as s_pool,
                    tc.tile_pool(name="psB", bufs=2, space="PSUM") as psum_pool,
                ):
                    th_sb = shard_pool.tile([P, tiles_per_core, dim], bf16)
                    spmm_phase(tc, oh_table[0:half, :], oh_table[half:npad, :], th_sb, False,
                               iota_t, meta_pool, g_pool, s_pool, psum_pool,
                               krep_phase=int(os.environ.get("KREPB", "0")) or None)
                    # interleave [oh | th | nv] per row so the query phase
                    # fetches all three with one 1536B-row gather (same bytes
                    # as separate gathers, 1/3 the descriptors).
                    nc.sync.dma_start(
                        comb_bounce[:, dim:2 * dim].rearrange("(t p) d -> p t d", p=P),
                        th_sb[:])
                nc.sync.dma_start(comb_bounce[:, 0:dim], oh_bounce[:])
                nc.sync.dma_start(comb_bounce[:, 2 * dim:3 * dim], nv_bounce[:])
                nc.gpsimd.collective_compute(
                    "AllGather", mybir.AluOpType.bypass,
                    replica_groups=[list(range(NCORES))],
                    ins=[comb_bounce[:]], outs=[comb_table[:]])
                if dbg_mode == "AB":
                    nc.sync.dma_start(dbg_d[:], comb_table[:, dim:2 * dim])

            # ---- phase C: query dots (no loops; whole-group tensors) ----
            if dbg_mode == "":
                with (
                    tc.tile_pool(name="qidx", bufs=2) as qidx_pool,
                    tc.tile_pool(name="qg", bufs=1) as qg_pool,
                    tc.tile_pool(name="qtmp", bufs=1) as qtmp_pool,
                    tc.tile_pool(name="qout", bufs=1) as qout_pool,
                ):
                    mul = mybir.AluOpType.mult
                    add = mybir.AluOpType.add
                    sub = mybir.AluOpType.subtract
                    X = mybir.AxisListType.X
                    nidx = ch * P
                    for _ in range(int(os.environ.get("KREPC", "0")) or krep):
                        for g in range(4):
                            s_lo = (g // 2) == 0
                            t_lo = (g % 2) == 0

                            def tab(t_, lo):
                                return t_[0:half, :] if lo else t_[half:npad, :]

                            idx_s = load_idx(qidx_pool, "qis", idx_reps["qidx_s"], g * ch * 8, ch * 8)
                            idx_t = load_idx(qidx_pool, "qit", idx_reps["qidx_t"], g * ch * 8, ch * 8)
                            sc_b = qidx_pool.tile([P, 4, ch], bf16, tag="scb", name="sc_b")
                            nc.sync.dma_start(sc_b[:], qsc_d[:, g * 4 * ch:(g + 1) * 4 * ch]
                                              .rearrange("p (j c) -> p j c", j=4))
                            cs_b = qidx_pool.tile([P, ch], f32, tag="csb", name="cs_b")
                            ct_b = qidx_pool.tile([P, ch], f32, tag="ctb", name="ct_b")
                            nc.vector.tensor_tensor(out=cs_b[:], in0=sc_b[:, 0, :],
                                                    in1=sc_b[:, 1, :], op=mul)
                            nc.vector.tensor_tensor(out=ct_b[:], in0=sc_b[:, 2, :],
                                                    in1=sc_b[:, 3, :], op=mul)

                            tiles = {}
                            for name, table, idxt in (
                                    ("cs", tab(comb_table, s_lo), idx_s),
                                    ("ct", tab(comb_table, t_lo), idx_t)):
                                t_ = qg_pool.tile([P, ch, 3 * dim], bf16, tag=name, name=name)
                                split_gather(t_, table, idxt, ch, 3 * dim)
                                tiles[name] = t_

                            acc = qout_pool.tile([P, 6, ch], f32, tag="acc", name="acc")
                            prod = qtmp_pool.tile([P, ch, dim], bf16, tag="prod", name="prod")
                            zs_t = qtmp_pool.tile([P, ch, dim], bf16, tag="zs", name="zs_t")
                            zt_t = qtmp_pool.tile([P, ch, dim], bf16, tag="zt", name="zt_t")

                            def bcast(t2d):
                                ap = t2d[:]
                                return bass.AP(ap.tensor, ap.offset,
                                               [ap.ap[0], [1, ch], [0, dim]])

                            def dot(dst_j, a_ap, b_ap):
                                nc.vector.tensor_tensor(out=prod[:], in0=a_ap, in1=b_ap, op=mul)
                                nc.vector.tensor_reduce(out=acc[:, dst_j, :], in_=prod[:],
                                                        axis=X, op=add)

                            ohs = tiles["cs"][:, :, 0:dim]
                            ths = tiles["cs"][:, :, dim:2 * dim]
                            nvs = tiles["cs"][:, :, 2 * dim:3 * dim]
                            oht = tiles["ct"][:, :, 0:dim]
                            tht = tiles["ct"][:, :, dim:2 * dim]
                            nvt = tiles["ct"][:, :, 2 * dim:3 * dim]
                            dot(0, ohs, oht)
                            dot(1, ohs, tht)
                            dot(2, ths, oht)
                            dot(4, ohs, ths)
                            dot(5, oht, tht)
                            # z = th - (deg*w) * nv
                            nc.vector.tensor_tensor(out=zs_t[:], in0=nvs,
                                                    in1=bcast(cs_b), op=mul)
                            nc.vector.tensor_tensor(out=zs_t[:], in0=ths, in1=zs_t[:], op=sub)
                            nc.vector.tensor_tensor(out=zt_t[:], in0=nvt,
                                                    in1=bcast(ct_b), op=mul)
                            nc.vector.tensor_tensor(out=zt_t[:], in0=tht, in1=zt_t[:], op=sub)
                            dot(3, zs_t[:], zt_t[:])
                            # c12 = acc1+acc2, cself = acc4+acc5
                            nc.vector.tensor_tensor(out=acc[:, 1, :], in0=acc[:, 1, :],
                                                    in1=acc[:, 2, :], op=add)
                            nc.vector.tensor_tensor(out=acc[:, 4, :], in0=acc[:, 4, :],
                                                    in1=acc[:, 5, :], op=add)
                            for jj, aj in enumerate((0, 1, 3, 4)):
                                nc.sync.dma_start(out_d[jj][:, g * ch:(g + 1) * ch],
                                                  acc[:, aj, :])

    nc.compile()
    return nc


def _prepare(edges, adj_row, adj_col, node_weight, node_vectors):
    edges = np.asarray(edges)
    adj_row = np.asarray(adj_row).astype(np.int64)
    adj_col = np.asarray(adj_col).astype(np.int64)
    node_weight = np.asarray(node_weight, dtype=np.float32)
    node_vectors = np.asarray(node_vectors, dtype=np.float32)

    n, dim = node_vectors.shape
    eq = edges.shape[1]
    s_nodes = np.asarray(edges[0]).astype(np.int64)
    t_nodes = np.asarray(edges[1]).astype(np.int64)

    tiles_per_core = -(-n // (NCORES * P))
    tiles_per_core += tiles_per_core % 2  # even, for pair-gathers
    shard = tiles_per_core * P
    npad = NCORES * shard
    half = npad // 2
    ntiles = NCORES * tiles_per_core
    assert half <= 32767, "table half must fit int16 gather indices"

    deg = np.bincount(adj_row, minlength=n).astype(np.float32)

    # degree-balanced relabeling: snake rows (sorted by degree desc) across
    # all tiles so each tile carries ~the same number of edges.
    order_rows = np.argsort(-deg, kind="stable")
    slot_ids = np.arange(npad)
    rounds = slot_ids // ntiles                    # 0..127 (= row slot in tile)
    pos = slot_ids % ntiles
    tiles_seq = np.where(rounds % 2 == 0, pos, ntiles - 1 - pos)
    new_ids_seq = tiles_seq * P + rounds           # new id for degree-rank r
    perm = np.full(npad, -1, np.int64)             # new_id -> old_id
    perm[new_ids_seq[:n]] = order_rows
    valid = perm >= 0
    pi = np.full(n, -1, np.int64)                  # old_id -> new_id
    pi[perm[valid]] = np.nonzero(valid)[0]

    # second pass: within each (round, table-half) the rows have ~equal total
    # degree, so permuting them across that half's tiles keeps tile totals
    # balanced while evening out each tile's lo/hi split (which otherwise
    # drifts binomially and costs a whole extra 128-slot gather chunk).
    is_lo_col0 = pi[adj_col] < half
    dlo = np.bincount(adj_row[is_lo_col0], minlength=n)
    htiles = ntiles // 2
    lo_load = np.zeros(ntiles, np.int64)
    perm2 = np.full(npad, -1, np.int64)
    for r in range(npad // ntiles):
        base = r * ntiles
        for hh in range(2):
            tset = np.arange(hh * htiles, (hh + 1) * htiles)
            slots = tset * P + r
            olds = perm[slots]
            ok = olds >= 0
            rdlo = np.where(ok, dlo[np.where(ok, olds, 0)], -1)
            row_order = np.argsort(-rdlo, kind="stable")
            tile_order = tset[np.argsort(lo_load[tset], kind="stable")]
            chosen = olds[row_order]
            dest = tile_order * P + r
            perm2[dest] = chosen
            okc = chosen >= 0
            lo_load[tile_order[okc]] += rdlo[row_order][okc]
    perm = perm2
    valid = perm >= 0
    pi = np.full(n, -1, np.int64)
    pi[perm[valid]] = np.nonzero(valid)[0]

    row_new = pi[adj_row]
    col_new = pi[adj_col]
    s_new = pi[s_nodes]
    t_new = pi[t_nodes]

    w_bf = node_weight.astype(ml_dtypes.bfloat16)
    nv_pad = np.zeros((npad, dim), ml_dtypes.bfloat16)
    nv_pad[valid] = node_vectors.astype(ml_dtypes.bfloat16)[perm[valid]]

    core_of = row_new // shard
    tile_of = (row_new % shard) // P
    rl_of = row_new % P
    is_lo = col_new < half

    key = core_of * tiles_per_core + tile_of
    cnt_lo = np.bincount(key[is_lo], minlength=ntiles)
    cnt_hi = np.bincount(key[~is_lo], minlength=ntiles)
    c_lo = max(1, int(-(-cnt_lo.max() // P)))
    c_hi = max(1, int(-(-cnt_hi.max() // P)))
    c_tot = c_lo + c_hi

    order = np.lexsort((~is_lo, tile_of, core_of))

    # ---- query groups ----
    q_core = np.repeat(np.arange(NCORES), -(-eq // NCORES))[:eq]
    q_group = np.where(s_new < half, 0, 2) + np.where(t_new < half, 0, 1)
    grp_cnt = np.zeros((NCORES, 4), np.int64)
    for k in range(NCORES):
        m = q_core == k
        grp_cnt[k] = np.bincount(q_group[m], minlength=4)
    ch = max(1, int(-(-grp_cnt.max() // P)))

    cache_key = (dim, npad, tiles_per_core, c_lo, c_hi, ch)
    if cache_key not in _CACHE:
        _CACHE[cache_key] = _build_program(dim, npad, tiles_per_core, c_lo, c_hi, ch)
    nc = _CACHE[cache_key]

    wcol_bf = w_bf[adj_col].astype(np.float32)
    deg_new = np.zeros(npad, np.float32)
    deg_new[valid] = deg[perm[valid]]
    w_new = np.zeros(npad, np.float32)
    w_new[valid] = w_bf[perm[valid]].astype(np.float32)

    in_maps = []
    q_positions = []
    for k in range(NCORES):
        sel = order[core_of[order] == k]
        idx_lo_arr = np.zeros((tiles_per_core, c_lo * P), np.int16)
        idx_hi_arr = np.zeros((tiles_per_core, c_hi * P), np.int16)
        rl_arr = np.full((P, tiles_per_core * c_tot), 255.0, np.float32)
        w_arr = np.zeros((P, tiles_per_core * c_tot), np.float32)
        for t in range(tiles_per_core):
            et = sel[tile_of[sel] == t]
            lo_e = et[is_lo[et]]
            hi_e = et[~is_lo[et]]
            nl, nh = len(lo_e), len(hi_e)
            idx_lo_arr[t, :nl] = col_new[lo_e]
            idx_hi_arr[t, :nh] = col_new[hi_e] - half
            slots = np.arange(nl)
            rl_arr[slots % P, t * c_tot + slots // P] = rl_of[lo_e]
            w_arr[slots % P, t * c_tot + slots // P] = wcol_bf[lo_e]
            slots = np.arange(nh)
            rl_arr[slots % P, t * c_tot + c_lo + slots // P] = rl_of[hi_e]
            w_arr[slots % P, t * c_tot + c_lo + slots // P] = wcol_bf[hi_e]

        idx_lo_w = np.concatenate([_wrap16(idx_lo_arr[t]) for t in range(tiles_per_core)], axis=1)
        idx_hi_w = np.concatenate([_wrap16(idx_hi_arr[t]) for t in range(tiles_per_core)], axis=1)

        qsel = np.nonzero(q_core == k)[0]
        qidx_s_arr = np.zeros((4, ch * P), np.int16)
        qidx_t_arr = np.zeros((4, ch * P), np.int16)
        qsc_arr = np.zeros((P, 4 * 4 * ch), np.float32)
        qpos = np.full((4, ch * P), -1, np.int64)
        for g in range(4):
            qg = qsel[q_group[qsel] == g]
            qg = qg[np.argsort(s_new[qg], kind="stable")]
            m = len(qg)
            sv = s_new[qg]
            tv = t_new[qg]
            qidx_s_arr[g, :m] = np.where(sv < half, sv, sv - half)
            qidx_t_arr[g, :m] = np.where(tv < half, tv, tv - half)
            qpos[g, :m] = qg
            slots = np.arange(m)
            pcol = (slots % P, slots // P)
            base = g * 4 * ch
            qsc_arr[pcol[0], base + pcol[1]] = deg_new[sv]
            qsc_arr[pcol[0], base + ch + pcol[1]] = w_new[sv]
            qsc_arr[pcol[0], base + 2 * ch + pcol[1]] = deg_new[tv]
            qsc_arr[pcol[0], base + 3 * ch + pcol[1]] = w_new[tv]

        qidx_s_w = np.concatenate([_wrap16(qidx_s_arr[g]) for g in range(4)], axis=1)
        qidx_t_w = np.concatenate([_wrap16(qidx_t_arr[g]) for g in range(4)], axis=1)

        in_maps.append({
            "nv": np.ascontiguousarray(nv_pad[k * shard:(k + 1) * shard]),
            "idx_lo": idx_lo_w,
            "idx_hi": idx_hi_w,
            "rl": rl_arr.astype(ml_dtypes.bfloat16),
            "w": w_arr.astype(ml_dtypes.bfloat16),
            "qidx_s": qidx_s_w,
            "qidx_t": qidx_t_w,
            "qsc": qsc_arr.astype(ml_dtypes.bfloat16),
        })
        q_positions.append(qpos)

    return nc, in_maps, q_positions, eq, ch


def kernel(edges, adj_row, adj_col, node_weight, node_vectors):
    _patch_cc_flags()
    nc, in_maps, q_positions, eq, ch = _prepare(
        edges, adj_row, adj_col, node_weight, node_vectors)
    res = run_bass_kernel_spmd(nc, in_maps, core_ids=list(range(NCORES)))
    outs = [res.results[k]["out"] for k in range(NCORES)]
    return _assemble(outs, q_positions, eq, ch)


def _assemble(outs, q_positions, eq, ch):
    counts = [np.zeros(eq, np.float32) for _ in range(4)]
    for k in range(NCORES):
        out = outs[k]  # [4, 128, 4*ch]
        for g in range(4):
            qpos = q_positions[k][g]
            slots = np.nonzero(qpos >= 0)[0]
            pp = slots % P
            cc = g * ch + slots // P
            for j in range(4):
                counts[j][qpos[slots]] = out[j, pp, cc]
    return tuple(counts)
